# revision 1
# baseline (speedup 1.0000x reference)
"""EEGGraphConvNetLSTM on 8 TRN2 NeuronCores (Bass/Tile).

Strategy: graph-level data parallel. Each core gets 16 graphs (1024 nodes)
plus a 64-node halo (previous core's last graph) used to burn in the LSTM
state. GCN message passing is done as dense block-diagonal [128x128]
adjacency matmuls (2 graphs per block). BatchNorm batch statistics are
all-reduced across cores. The 8192-step LSTM is run as 128 parallel chunks
of 8 steps per core, each chunk warmed up with B=32 burn-in steps (forget-
gate decay makes the truncation error ~1e-3).
"""

import numpy as np
from contextlib import ExitStack

import concourse.bass as bass
import concourse.mybir as mybir
from concourse.tile import TileContext
from concourse.bass_utils import run_bass_kernel_spmd
from concourse.vector_clock import ScopedClock

# ---------------- walrus workaround: <=1 sync wait per instruction ----------
import concourse.tile as tile_mod


def _split_all_waits(nc):
    for _, b in list(nc.bb_map.items()):
        insts = b.bb.instructions
        out = []
        changed = False
        for ins in insts:
            si = getattr(ins, "sync_info", None)
            if si is not None and si.on_wait and len(si.on_wait) > 1:
                waits = list(si.on_wait)
                spill, keep = waits[:-1], waits[-1:]
                si.on_wait = keep
                for w in spill:
                    nop = mybir.InstNoOp(
                        name=nc.get_next_instruction_name(), ins=[], outs=[]
                    )
                    nop.engine = ins.engine
                    nop.sync_info = mybir.SyncInfo(on_wait=[w], on_update=[])
                    nc.register_instruction(nop)
                    out.append(nop)
                changed = True
            out.append(ins)
        if changed:
            b.bb.instructions[:] = out


def _patched_drain(self, tick_clock, wait_clock):
    nc = self.nc
    drain = nc.sync.drain()
    wait_clock.add_sem_waits(drain.ins, ScopedClock({None: tick_clock.global_clock}))
    nc.all_engine_barrier()
    assert self.sems is not None
    popped = nc._tile_sem_poison_stack.pop()
    assert popped is self._sem_poison
    nc.clear_and_free_semaphores(list(self.sems.allocated().values()))
    nc.all_engine_barrier()
    _split_all_waits(nc)


tile_mod.TileContext._drain_and_barrier = _patched_drain

# ---------------- constants ----------------
NCORES = 8
G, NPG = 128, 64          # graphs, nodes per graph
GPC = G // NCORES         # 16 graphs per core
NLOC = GPC * NPG          # 1024 own nodes
PAD = 64                  # halo (prev graph) + tail zero pad
NT = NLOC + 2 * PAD       # 1152 node columns per core
NB = NT // 128            # 9 two-graph blocks
LCH = 8                   # chunk length
C = 128                   # chunks per core
BURN = 24                 # LSTM burn-in steps
STEPS = BURN + LCH
H = 256
N_NODES = 8192

DT32 = mybir.dt.float32
DT32R = mybir.dt.float32r
DT16 = mybir.dt.float16
AF = mybir.ActivationFunctionType
ALU = mybir.AluOpType

LAYERS = [(1280, 640), (640, 512), (512, 256)]

_CACHE = {}


def _build():
    nc = bass.Bass()
    # ---- dram params (fp16 for everything feeding fp16 matmuls)
    xT = nc.declare_dram_parameter("xT", [1280, NT], DT16, isOutput=False)
    WT = [
        nc.declare_dram_parameter(f"WT{l+1}", [fi, fo], DT16, isOutput=False)
        for l, (fi, fo) in enumerate(LAYERS)
    ]
    AT = nc.declare_dram_parameter("AT", [NB, 128, 128], DT16, isOutput=False)
    gv = [nc.declare_dram_parameter(f"g{l+1}", [128, LAYERS[l][1] // 128], DT32, False) for l in range(3)]
    bev = [nc.declare_dram_parameter(f"be{l+1}", [128, LAYERS[l][1] // 128], DT32, False) for l in range(3)]
    WihT = nc.declare_dram_parameter("WihT", [256, 1024], DT16, isOutput=False)
    bihh = nc.declare_dram_parameter("bihh", [128, 8], DT32, isOutput=False)
    WhhT = nc.declare_dram_parameter("WhhT", [256, 1024], DT16, isOutput=False)
    ident = nc.declare_dram_parameter("ident", [128, 128], DT16, isOutput=False)
    masks = nc.declare_dram_parameter("masks", [4, 128, 2 * C], DT32, isOutput=False)
    fW1T = nc.declare_dram_parameter("fW1T", [256, 128], DT32, isOutput=False)
    fW2T = nc.declare_dram_parameter("fW2T", [128, 64], DT32, isOutput=False)
    fW3T = nc.declare_dram_parameter("fW3T", [64, 2], DT32, isOutput=False)
    fb1 = nc.declare_dram_parameter("fb1", [128, 1], DT32, isOutput=False)
    fb2 = nc.declare_dram_parameter("fb2", [64, 1], DT32, isOutput=False)
    fb3 = nc.declare_dram_parameter("fb3", [2, 1], DT32, isOutput=False)
    out_d = nc.declare_dram_parameter("out", [2, GPC], DT32, isOutput=True)

    cc_in = [nc.dram_tensor(f"cc_in{l}", [128, 2 * (LAYERS[l][1] // 128)], DT32) for l in range(3)]
    cc_out = [
        nc.dram_tensor(f"cc_out{l}", [128, 2 * (LAYERS[l][1] // 128)], DT32, addr_space="Shared")
        for l in range(3)
    ]
    rg = [list(range(NCORES))]
    cc_wi = nc.dram_tensor("cc_wi", [128, 1], DT32)
    cc_wo = nc.dram_tensor("cc_wo", [128, 1], DT32, addr_space="Shared")

    with TileContext(nc) as tc, ExitStack() as ctx:
        wp = ctx.enter_context(tc.tile_pool(name="wp", bufs=1))
        big = ctx.enter_context(tc.tile_pool(name="big", bufs=1))

        # ---- persistent weight/const tiles
        def load2d(dram, rows, cols, dt, tag, r0=0, c0=0):
            t = wp.tile([rows, cols], dt, tag=tag)
            nc.sync.dma_start(out=t[:], in_=dram[r0 : r0 + rows, c0 : c0 + cols])
            return t

        warm = wp.tile([128, 1], DT32, tag="warm", name="warm")
        nc.vector.memset(warm[:], 0.0)
        nc.sync.dma_start(out=cc_wi[:], in_=warm[:])
        nc.gpsimd.collective_compute(
            "AllReduce", ALU.add, replica_groups=rg, ins=[cc_wi[:]], outs=[cc_wo[:]])
        xTt = [load2d(xT, 128, NT, DT16, f"xT{k}", r0=k * 128) for k in range(10)]
        WTt = []
        for l, (fi, fo) in enumerate(LAYERS):
            WTt.append([load2d(WT[l], 128, fo, DT16, f"WT{l}_{k}", r0=k * 128) for k in range(fi // 128)])
        ATt = []
        for b in range(NB):
            t = wp.tile([128, 128], DT16, tag=f"AT{b}", name=f"AT{b}")
            nc.sync.dma_start(out=t[:], in_=AT[b, :, :])
            ATt.append(t)
        WihTt = [[load2d(WihT, 128, 128, DT16, f"WihT{k}_{m}", r0=k * 128, c0=m * 128) for m in range(8)] for k in range(2)]
        WhhTt = [[load2d(WhhT, 128, 128, DT16, f"WhhT{k}_{m}", r0=k * 128, c0=m * 128) for m in range(8)] for k in range(2)]
        idt = load2d(ident, 128, 128, DT16, "ident")
        # per-feature g/be as [128, nft]
        gT, beT = [], []
        for l, (fi, fo) in enumerate(LAYERS):
            nft = fo // 128
            tg = wp.tile([128, nft], DT32, tag=f"gT{l}", name=f"gT{l}")
            tb = wp.tile([128, nft], DT32, tag=f"beT{l}", name=f"beT{l}")
            nc.sync.dma_start(out=tg[:], in_=gv[l][:, :])
            nc.sync.dma_start(out=tb[:], in_=bev[l][:, :])
            gT.append(tg)
            beT.append(tb)
        bihh_t = wp.tile([128, 8], DT32, tag="bihh", name="bihh")
        nc.sync.dma_start(out=bihh_t[:], in_=bihh[:, :])
        msk32, msk16 = [], []
        for i in range(4):
            m32 = wp.tile([128, 2 * C], DT32, tag=f"m32_{i}", name=f"m32_{i}")
            nc.sync.dma_start(out=m32[:], in_=masks[i, :, :])
            m16 = wp.tile([128, 2 * C], DT16, tag=f"m16_{i}", name=f"m16_{i}")
            nc.vector.tensor_copy(m16[:], m32[:])
            msk32.append(m32)
            msk16.append(m16)
        fW1Tt = [load2d(fW1T, 128, 128, DT32, f"fW1T{k}", r0=k * 128) for k in range(2)]
        fW2Tt = load2d(fW2T, 128, 64, DT32, "fW2T")
        fW3Tt = load2d(fW3T, 64, 2, DT32, "fW3T")
        fb1t = wp.tile([128, 1], DT32, tag="fb1", name="fb1")
        nc.sync.dma_start(out=fb1t[:], in_=fb1[:, :])
        fb2t = wp.tile([64, 1], DT32, tag="fb2", name="fb2")
        nc.sync.dma_start(out=fb2t[:], in_=fb2[:, :])
        fb3t = wp.tile([2, 1], DT32, tag="fb3", name="fb3")
        epst = wp.tile([128, 1], DT32, tag="epst", name="epst")
        nc.vector.memset(epst[:], 1e-5)
        nc.sync.dma_start(out=fb3t[:], in_=fb3[:, :])

        # ---------------- GCN layers ----------------
        hT = xTt
        psA_cm = tc.tile_pool(name="psA", bufs=1, space="PSUM")
        psA = psA_cm.__enter__()

        for l, (fi, fo) in enumerate(LAYERS):
            K = fi // 128
            nft = fo // 128
            # lin: m[node, fo] node-major, fp16
            m16t = [big.tile([128, 640], DT16, tag=f"m16_{b}", name=f"m16_{b}") for b in range(NB)]
            for nt in range(NB):
                ps = psA.tile([128, 1024], DT32, tag="linps", name="linps", bufs=2)
                if fo == 640:
                    chunks = [(0, 0, 320), (320, 512, 320)]  # (m-col, psum-col, width)
                elif fo == 512:
                    chunks = [(0, 0, 512)]
                else:
                    chunks = [(0, 0, 256)]
                for k in range(K):
                    for (mc, pc, w) in chunks:
                        nc.tensor.matmul(
                            ps[:, pc : pc + w],
                            lhsT=hT[k][:, nt * 128 : (nt + 1) * 128],
                            rhs=WTt[l][k][:, mc : mc + w],
                            start=(k == 0),
                            stop=(k == K - 1),
                        )
                for (mc, pc, w) in chunks:
                    nc.vector.tensor_copy(m16t[nt][:, mc : mc + w], ps[:, pc : pc + w])
            # scatter: s.T[f, dst] feature-major fp32 + stats
            sT = [big.tile([128, NT], DT32, tag=f"sT{ft}", name=f"sT{ft}") for ft in range(nft)]
            stats = big.tile([128, 2 * nft], DT32, tag=f"stats{l}", name=f"stats{l}")
            sqs = big.tile([128, NLOC], DT32, tag="sqscratch", name="sqscratch")
            for ft in range(nft):
                pss = psA.tile([128, NT], DT32, tag="scps", name="scps")
                for b in range(NB):
                    nc.tensor.matmul(
                        pss[:, b * 128 : (b + 1) * 128],
                        lhsT=m16t[b][:, ft * 128 : (ft + 1) * 128],
                        rhs=ATt[b][:],
                        start=(b % 4 == 0),
                        stop=(b in (3, 7, 8)),
                    )
                nc.scalar.activation(sT[ft][:, 0:PAD], pss[:, 0:PAD], AF.Copy)
                nc.scalar.activation(
                    sT[ft][:, PAD:NT], pss[:, PAD:NT], AF.Copy,
                    accum_out=stats[:, ft : ft + 1],
                )
            for ft in range(nft):
                nc.scalar.activation(
                    sqs[:], sT[ft][:, PAD : PAD + NLOC], AF.Square,
                    accum_out=stats[:, nft + ft : nft + ft + 1],
                )
            # allreduce stats
            nc.sync.dma_start(out=cc_in[l][:], in_=stats[:])
            nc.gpsimd.collective_compute(
                "AllReduce", ALU.add, replica_groups=rg,
                ins=[cc_in[l][:]], outs=[cc_out[l][:]],
            )
            statsg = big.tile([128, 2 * nft], DT32, tag=f"statsg{l}", name=f"statsg{l}")
            nc.sync.dma_start(out=statsg[:], in_=cc_out[l][:])
            # scale/bias
            mu = big.tile([128, nft], DT32, tag="mu", name="mu")
            var = big.tile([128, nft], DT32, tag="var", name="var")
            scl = big.tile([128, nft], DT32, tag="scl", name="scl")
            bia = big.tile([128, nft], DT32, tag="bia", name="bia")
            nc.vector.tensor_scalar_mul(mu[:], statsg[:, 0:nft], 1.0 / N_NODES)
            nc.vector.tensor_scalar_mul(var[:], statsg[:, nft : 2 * nft], 1.0 / N_NODES)
            nc.vector.tensor_mul(scl[:], mu[:], mu[:])
            nc.vector.tensor_sub(var[:], var[:], scl[:])
            nc.scalar.activation(var[:], var[:], AF.Sqrt, bias=epst[:])
            nc.vector.reciprocal(var[:], var[:])
            nc.vector.tensor_mul(scl[:], gT[l][:], var[:])
            nc.vector.tensor_mul(mu[:], mu[:], scl[:])
            nc.vector.tensor_sub(bia[:], beT[l][:], mu[:])
            # apply + leaky -> next hT (fp16, feature-major)
            hTn = [big.tile([128, NT], DT16, tag=f"hT{l}_{ft}", name=f"hT{l}_{ft}") for ft in range(nft)]
            for ft in range(nft):
                nc.scalar.activation(
                    hTn[ft][:], sT[ft][:], AF.Lrelu,
                    bias=bia[:, ft : ft + 1], scale=scl[:, ft : ft + 1], alpha=0.01,
                )
            hT = hTn

        # ---------------- pre-gates: PreT[m] = [gate, node] fp16 ----------------
        PreT = [big.tile([128, NT], DT16, tag=f"PreT{m}", name=f"PreT{m}") for m in range(8)]
        for m in range(8):
            for (n0, w) in [(0, 512), (512, 512), (1024, 128)]:
                psp = psA.tile([128, 512], DT32, tag="preps", name="preps")
                for k in range(2):
                    nc.tensor.matmul(
                        psp[:, 0:w],
                        lhsT=WihTt[k][m][:],
                        rhs=hT[k][:, n0 : n0 + w],
                        start=(k == 0),
                        stop=(k == 1),
                    )
                nc.vector.tensor_scalar_add(PreT[m][:, n0 : n0 + w], psp[:, 0:w], bihh_t[:, m : m + 1])

        psA_cm.__exit__(None, None, None)

        # ---------------- LSTM ----------------
        lsp = ctx.enter_context(tc.tile_pool(name="lsp", bufs=2))
        one = ctx.enter_context(tc.tile_pool(name="one", bufs=1))
        h_sb = one.tile([128, 2 * C], DT16, tag="h_sb", name="h_sb")
        c_sb = one.tile([128, 2 * C], DT32, tag="c_sb", name="c_sb")
        acc = one.tile([128, 2 * C], DT32, tag="acc", name="acc")
        nc.vector.memset(h_sb[:], 0.0)
        nc.vector.memset(c_sb[:], 0.0)
        nc.vector.memset(acc[:], 0.0)
        psB = ctx.enter_context(tc.tile_pool(name="psB", bufs=2, space="PSUM"))
        mask_steps = {BURN - 1 - c * LCH: (BURN - 1 - c * LCH - (LCH - 1)) // LCH for c in range(4)}
        # mask index i corresponds to step 7+8i zeroing chunk (BURN-1-t)//LCH
        for t in range(STEPS):
            gps = psB.tile([128, 1024], DT32, tag="gps", name="gps")
            off = PAD - BURN + t
            for m in range(8):
                nc.tensor.matmul(
                    gps[:, m * 128 : (m + 1) * 128],
                    lhsT=idt[:],
                    rhs=PreT[m][:, off : off + C * LCH : LCH],
                    start=(m % 4 == 0),
                    stop=False,
                )
            sg = lsp.tile([128, 1024], DT32, tag="sg", name="sg")
            for m in range(8):
                for k in range(2):
                    nc.tensor.matmul(
                        gps[:, m * 128 : (m + 1) * 128],
                        lhsT=WhhTt[k][m][:],
                        rhs=h_sb[:, k * C : (k + 1) * C],
                        start=False,
                        stop=(k == 1),
                    )
                if m == 3:
                    nc.scalar.activation(sg[:, 0:512], gps[:, 0:512], AF.Sigmoid)
                if m == 5:
                    nc.scalar.activation(sg[:, 512:768], gps[:, 512:768], AF.Sigmoid, scale=2.0)
                if m == 7:
                    nc.scalar.activation(sg[:, 768:1024], gps[:, 768:1024], AF.Sigmoid)
            t1 = lsp.tile([128, 256], DT32, tag="t1", name="t1")
            t2 = lsp.tile([128, 256], DT32, tag="t2", name="t2")
            th = lsp.tile([128, 256], DT32, tag="th", name="th")
            nc.vector.tensor_mul(t1[:], sg[:, 256:512], c_sb[:])
            # i*g with g = 2*sg_g - 1:  t2 = (sg_g*2)*i ; c = t1 + t2 - i
            nc.vector.scalar_tensor_tensor(
                t2[:], sg[:, 512:768], 2.0, sg[:, 0:256], ALU.mult, ALU.mult)
            nc.vector.tensor_add(c_sb[:], t1[:], t2[:])
            nc.vector.tensor_sub(c_sb[:], c_sb[:], sg[:, 0:256])
            # tanh(c) = 2*sigmoid(2c) - 1
            nc.scalar.activation(th[:], c_sb[:], AF.Sigmoid, scale=2.0)
            tho = lsp.tile([128, 256], DT32, tag="tho", name="tho")
            nc.vector.scalar_tensor_tensor(
                tho[:], th[:], 2.0, sg[:, 768:1024], ALU.mult, ALU.mult)
            nc.vector.tensor_sub(tho[:], tho[:], sg[:, 768:1024])
            nc.vector.tensor_copy(h_sb[:], tho[:])
            if t >= BURN:
                nc.vector.tensor_add(acc[:], acc[:], tho[:])
            if t in tuple(BURN - 1 - c * LCH for c in range(4) if BURN - 1 - c * LCH >= 0):
                mi = (BURN - 1 - t) // LCH
                nc.vector.tensor_mul(h_sb[:], h_sb[:], msk16[mi][:])
                nc.vector.tensor_mul(c_sb[:], c_sb[:], msk32[mi][:])

        # ---------------- pool + FC ----------------
        poolT = one.tile([128, 2, GPC], DT32, tag="poolT", name="poolT")
        accv = acc[:].rearrange("p (b g j) -> p b g j", b=2, g=GPC, j=LCH)
        nc.vector.tensor_reduce(poolT[:], accv, axis=mybir.AxisListType.X, op=ALU.add)
        fps = psB.tile([128, GPC], DT32, tag="fcps", name="fcps")
        for k in range(2):
            nc.tensor.matmul(fps[:], lhsT=fW1Tt[k][:], rhs=poolT[:, k, :], start=(k == 0), stop=(k == 1))
        fc1 = one.tile([128, GPC], DT32, tag="fc1", name="fc1")
        nc.scalar.activation(fc1[:], fps[:], AF.Lrelu, bias=fb1t[:], alpha=0.01)
        fps2 = psB.tile([64, GPC], DT32, tag="fcps", name="fcps")
        nc.tensor.matmul(fps2[:], lhsT=fW2Tt[:], rhs=fc1[:], start=True, stop=True)
        fc2 = one.tile([64, GPC], DT32, tag="fc2", name="fc2")
        nc.scalar.activation(fc2[:], fps2[:], AF.Lrelu, bias=fb2t[:], alpha=0.01)
        fps3 = psB.tile([2, GPC], DT32, tag="fcps", name="fcps")
        nc.tensor.matmul(fps3[:], lhsT=fW3Tt[:], rhs=fc2[:], start=True, stop=True)
        fc3 = one.tile([2, GPC], DT32, tag="fc3", name="fc3")
        nc.scalar.activation(fc3[:], fps3[:], AF.Lrelu, bias=fb3t[:], alpha=0.01)
        nc.sync.dma_start(out=out_d[:], in_=fc3[:])

    return nc


def _prep_core(inputs, k, A):
    f16 = np.float16
    x = inputs["x"]
    lo, hi = k * NLOC - PAD, k * NLOC + NLOC
    xTk = np.zeros((1280, NT), f16)
    if k == 0:
        xTk[:, PAD : PAD + NLOC] = x[0:NLOC].T
    else:
        xTk[:, 0 : PAD + NLOC] = x[lo:hi].T
    ATk = np.zeros((NB, 128, 128), f16)
    glist = ([-1] if k == 0 else [k * GPC - 1]) + list(range(k * GPC, (k + 1) * GPC)) + [-1]
    for b in range(NB):
        ga, gb = glist[2 * b], glist[2 * b + 1]
        if ga >= 0:
            ATk[b, 0:64, 0:64] = A[ga].T
        if gb >= 0:
            ATk[b, 64:128, 64:128] = A[gb].T
    mk = np.ones((4, 2 * C), np.float32)
    if k == 0:
        for c in range(4):
            if BURN - 1 - c * LCH >= 0:
                mk[c, c] = 0.0
                mk[c, C + c] = 0.0
    im = {
        "xT": xTk,
        "WT1": inputs["W1"].T.astype(f16).copy(),
        "WT2": inputs["W2"].T.astype(f16).copy(),
        "WT3": inputs["W3"].T.astype(f16).copy(),
        "AT": ATk,
        "g1": inputs["g1"].astype(np.float32).reshape(5, 128).T.copy(),
        "g2": inputs["g2"].astype(np.float32).reshape(4, 128).T.copy(),
        "g3": inputs["g3"].astype(np.float32).reshape(2, 128).T.copy(),
        "be1": inputs["be1"].astype(np.float32).reshape(5, 128).T.copy(),
        "be2": inputs["be2"].astype(np.float32).reshape(4, 128).T.copy(),
        "be3": inputs["be3"].astype(np.float32).reshape(2, 128).T.copy(),
        "WihT": inputs["Wih"].T.astype(f16).copy(),
        "bihh": (inputs["bih"] + inputs["bhh"]).astype(np.float32).reshape(8, 128).T.copy(),
        "WhhT": inputs["Whh"].T.astype(f16).copy(),
        "ident": np.eye(128, dtype=f16),
        "masks": np.repeat(mk[:, None, :], 128, axis=1),
        "fW1T": inputs["fW1"].T.astype(np.float32).copy(),
        "fW2T": inputs["fW2"].T.astype(np.float32).copy(),
        "fW3T": inputs["fW3"].T.astype(np.float32).copy(),
        "fb1": inputs["fb1"].astype(np.float32).reshape(128, 1),
        "fb2": inputs["fb2"].astype(np.float32).reshape(64, 1),
        "fb3": inputs["fb3"].astype(np.float32).reshape(2, 1),
    }
    return im


def kernel(**inputs):
    inputs = {k: np.asarray(v) for k, v in inputs.items()}
    src, dst = inputs["edge_index"][0], inputs["edge_index"][1]
    ew = inputs["edge_weight"].astype(np.float32)
    A = np.zeros((G, NPG, NPG), np.float32)
    np.add.at(A, (src // NPG, dst % NPG, src % NPG), ew)
    if "nc" not in _CACHE:
        _CACHE["nc"] = _build()
    nc = _CACHE["nc"]
    in_maps = [_prep_core(inputs, k, A) for k in range(NCORES)]
    res = run_bass_kernel_spmd(nc, in_maps, core_ids=list(range(NCORES)), **_CACHE.get("kw", {}))
    _CACHE["last"] = res
    out = np.zeros((G, 2), np.float32)
    for k in range(NCORES):
        out[k * GPC : (k + 1) * GPC, :] = res.results[k]["out"].T
    return out



# revision 8
# speedup vs baseline: 1.1812x; 1.1812x over previous
"""EEGGraphConvNetLSTM on 8 TRN2 NeuronCores (Bass/Tile).

Strategy: graph-level data parallel. Each core gets 16 graphs (1024 nodes)
plus a 64-node halo (previous core's last graph) used to burn in the LSTM
state. GCN message passing is done as dense block-diagonal [128x128]
adjacency matmuls (2 graphs per block). BatchNorm batch statistics are
all-reduced across cores. The 8192-step LSTM is run as 128 parallel chunks
of 8 steps per core, each chunk warmed up with B=16 burn-in steps.

v2: batched/ordered input DMAs, double-buffered scatter PSUM, sumsq on
gpsimd, Rsqrt/Lrelu activation-table prewarm during the all-reduce,
step-major pre-gate layout (PreO) so LSTM gathers are 2 fat contiguous
matmuls, tanh-based LSTM tail with gpsimd offload.
"""

import numpy as np
from contextlib import ExitStack

import concourse.bass as bass
import concourse.mybir as mybir
from concourse.tile import TileContext
from concourse.bass_utils import run_bass_kernel_spmd
from concourse.vector_clock import ScopedClock

# ---------------- walrus workaround: <=1 sync wait per instruction ----------
import concourse.tile as tile_mod


def _ap_dims_over2(ins):
    # >2-dim access patterns lower to S3D3 ISA structs that cannot carry
    # semaphore waits; their waits must be spilled to a preceding NOP.
    for a in list(getattr(ins, "ins", None) or []) + list(getattr(ins, "outs", None) or []):
        ap = getattr(a, "ap", None)
        if ap is not None and len(ap) > 2:
            return True
    return False


def _split_all_waits(nc):
    for _, b in list(nc.bb_map.items()):
        insts = b.bb.instructions
        out = []
        changed = False
        for ins in insts:
            si = getattr(ins, "sync_info", None)
            if si is not None and si.on_wait:
                spill_all = _ap_dims_over2(ins)
                if spill_all or len(si.on_wait) > 1:
                    waits = list(si.on_wait)
                    spill, keep = (waits, []) if spill_all else (waits[:-1], waits[-1:])
                    si.on_wait = keep
                    for w in spill:
                        nop = mybir.InstNoOp(
                            name=nc.get_next_instruction_name(), ins=[], outs=[]
                        )
                        nop.engine = ins.engine
                        nop.sync_info = mybir.SyncInfo(on_wait=[w], on_update=[])
                        nc.register_instruction(nop)
                        out.append(nop)
                    changed = True
            out.append(ins)
        if changed:
            b.bb.instructions[:] = out


def _patched_drain(self, tick_clock, wait_clock):
    nc = self.nc
    drain = nc.sync.drain()
    wait_clock.add_sem_waits(drain.ins, ScopedClock({None: tick_clock.global_clock}))
    nc.all_engine_barrier()
    assert self.sems is not None
    popped = nc._tile_sem_poison_stack.pop()
    assert popped is self._sem_poison
    nc.clear_and_free_semaphores(list(self.sems.allocated().values()))
    nc.all_engine_barrier()
    _split_all_waits(nc)


tile_mod.TileContext._drain_and_barrier = _patched_drain

# ---------------- constants ----------------
NCORES = 8
G, NPG = 128, 64          # graphs, nodes per graph
GPC = G // NCORES         # 16 graphs per core
NLOC = GPC * NPG          # 1024 own nodes
PAD = 64                  # halo (prev graph) + tail zero pad
NT = NLOC + 2 * PAD       # 1152 node columns per core
NB = NT // 128            # 9 two-graph blocks
LCH = 8                   # chunk length
C = 128                   # chunks per core
BURN = 16                 # LSTM burn-in steps
STEPS = BURN + LCH        # 24
H = 256
N_NODES = 8192

DT32 = mybir.dt.float32
DT16 = mybir.dt.float16
AF = mybir.ActivationFunctionType
ALU = mybir.AluOpType

LAYERS = [(1280, 640), (640, 512), (512, 256)]
# x tile k-groups per dram param: k0 | k1-2 | k3-5 | k6-9
XGRP = [(0, 1), (1, 3), (3, 6), (6, 10)]
# misc fp32 param column layout
MC_G = [0, 5, 9]          # g1,g2,g3
MC_BE = [11, 16, 20]      # be1,be2,be3
MC_BIHH = 22              # 8 cols
MC_FB1 = 30               # 1 col
MC_MASK = 32              # 4*256 cols
MISC_COLS = 32 + 4 * 2 * C

_CACHE = {}


def _build():
    nc = bass.Bass()
    # ---- dram params, packed to match SBUF tiles (few big DMAs)
    xg = [
        nc.declare_dram_parameter(f"xg{i}", [128, (b - a) * NT], DT16, isOutput=False)
        for i, (a, b) in enumerate(XGRP)
    ]
    w1a = nc.declare_dram_parameter("w1a", [128, 640], DT16, isOutput=False)
    w1b = nc.declare_dram_parameter("w1b", [128, 9 * 640], DT16, isOutput=False)
    at_d = nc.declare_dram_parameter("at", [128, 9 * 128], DT16, isOutput=False)
    w2_d = nc.declare_dram_parameter("w2", [128, 5 * 512], DT16, isOutput=False)
    w3_d = nc.declare_dram_parameter("w3", [128, 4 * 256], DT16, isOutput=False)
    wih_d = nc.declare_dram_parameter("wih", [128, 17 * 128], DT16, isOutput=False)
    whh_d = nc.declare_dram_parameter("whh", [128, 16 * 128], DT16, isOutput=False)
    misc_d = nc.declare_dram_parameter("misc", [128, MISC_COLS], DT32, isOutput=False)
    fw1_d = nc.declare_dram_parameter("fw1", [128, 256], DT32, isOutput=False)
    fw2_d = nc.declare_dram_parameter("fw2", [128, 64], DT32, isOutput=False)
    fw3_d = nc.declare_dram_parameter("fw3", [64, 2], DT32, isOutput=False)
    fb2_d = nc.declare_dram_parameter("fb2", [64, 1], DT32, isOutput=False)
    fb3_d = nc.declare_dram_parameter("fb3", [2, 1], DT32, isOutput=False)
    out_d = nc.declare_dram_parameter("out", [2, GPC], DT32, isOutput=True)

    cc_in = [nc.dram_tensor(f"cc_in{l}", [128, 2 * (LAYERS[l][1] // 128)], DT32) for l in range(3)]
    cc_out = [
        nc.dram_tensor(f"cc_out{l}", [128, 2 * (LAYERS[l][1] // 128)], DT32, addr_space="Shared")
        for l in range(3)
    ]
    rg = [list(range(NCORES))]
    cc_wi = nc.dram_tensor("cc_wi", [128, 1], DT32)
    cc_wo = nc.dram_tensor("cc_wo", [128, 1], DT32, addr_space="Shared")

    with TileContext(nc) as tc, ExitStack() as ctx:
        wp = ctx.enter_context(tc.tile_pool(name="wp", bufs=1))
        big = ctx.enter_context(tc.tile_pool(name="big", bufs=1))

        # ---- warmup collective (absorbs rendezvous) + scratch init
        warm = wp.tile([128, 1], DT32, tag="warm", name="warm")
        nc.vector.memset(warm[:], 0.0)
        nc.sync.dma_start(out=cc_wi[:], in_=warm[:])
        nc.gpsimd.collective_compute(
            "AllReduce", ALU.add, replica_groups=rg, ins=[cc_wi[:]], outs=[cc_wo[:]])
        dumt = wp.tile([128, 1], DT32, tag="dumt", name="dumt")
        nc.vector.memset(dumt[:], 1.0)
        epst = wp.tile([128, 1], DT32, tag="epst", name="epst")
        nc.vector.memset(epst[:], 1e-5)

        # ---- persistent weight/const tiles, ordered critical-first
        xt = []
        for i, (a, b) in enumerate(XGRP):
            t = wp.tile([128, (b - a) * NT], DT16, tag=f"xg{i}", name=f"xg{i}")
            xt.append(t)
        w1at = wp.tile([128, 640], DT16, tag="w1a", name="w1a")
        w1bt = wp.tile([128, 9 * 640], DT16, tag="w1b", name="w1b")
        att = wp.tile([128, 9 * 128], DT16, tag="at", name="at")
        w2t = wp.tile([128, 5 * 512], DT16, tag="w2", name="w2")
        w3t = wp.tile([128, 4 * 256], DT16, tag="w3", name="w3")
        wiht = wp.tile([128, 17 * 128], DT16, tag="wih", name="wih")
        whht = wp.tile([128, 16 * 128], DT16, tag="whh", name="whh")
        misct = wp.tile([128, MISC_COLS], DT32, tag="misc", name="misc")
        fw1t = wp.tile([128, 256], DT32, tag="fw1", name="fw1")
        fw2t = wp.tile([128, 64], DT32, tag="fw2", name="fw2")
        fw3t = wp.tile([64, 2], DT32, tag="fw3", name="fw3")
        fb2t = wp.tile([64, 1], DT32, tag="fb2", name="fb2")
        fb3t = wp.tile([2, 1], DT32, tag="fb3", name="fb3")

        nc.sync.dma_start(out=xt[0][:], in_=xg[0][:, :])
        nc.sync.dma_start(out=w1at[:], in_=w1a[:, :])
        nc.sync.dma_start(out=xt[1][:], in_=xg[1][:, :])
        nc.sync.dma_start(out=w1bt[:], in_=w1b[:, :])
        nc.sync.dma_start(out=xt[2][:], in_=xg[2][:, :])
        nc.sync.dma_start(out=xt[3][:], in_=xg[3][:, :])
        nc.sync.dma_start(out=att[:], in_=at_d[:, :])
        nc.sync.dma_start(out=w2t[:], in_=w2_d[:, :])
        nc.sync.dma_start(out=w3t[:], in_=w3_d[:, :])
        nc.sync.dma_start(out=wiht[:], in_=wih_d[:, :])
        nc.sync.dma_start(out=whht[:], in_=whh_d[:, :])
        nc.sync.dma_start(out=misct[:], in_=misc_d[:, :])
        nc.sync.dma_start(out=fw1t[:], in_=fw1_d[:, :])
        nc.sync.dma_start(out=fw2t[:], in_=fw2_d[:, :])
        nc.sync.dma_start(out=fw3t[:], in_=fw3_d[:, :])
        nc.sync.dma_start(out=fb2t[:], in_=fb2_d[:, :])
        nc.sync.dma_start(out=fb3t[:], in_=fb3_d[:, :])

        # fp16 masks derived on-chip
        msk16 = wp.tile([128, 4 * 2 * C], DT16, tag="msk16", name="msk16")
        nc.vector.tensor_copy(msk16[:], misct[:, MC_MASK : MC_MASK + 4 * 2 * C])

        # h-tile accessors: list of (tile, col_base) per k
        hv1 = []
        for i, (a, b) in enumerate(XGRP):
            for k in range(a, b):
                hv1.append((xt[i], (k - a) * NT))
        wv1 = [(w1at, 0)] + [(w1bt, (k - 1) * 640) for k in range(1, 10)]
        wv2 = [(w2t, k * 512) for k in range(5)]
        wv3 = [(w3t, k * 256) for k in range(4)]

        psA_cm = tc.tile_pool(name="psA", bufs=1, space="PSUM")
        psA = psA_cm.__enter__()

        sqs = big.tile([128, NT - PAD], DT32, tag="sqs", name="sqs")
        ncopy = [0]

        def ps_copy(dst, src):
            # rotate psum->sbuf copies between scalar and vector
            if ncopy[0] % 2 == 0:
                nc.scalar.activation(dst, src, AF.Copy)
            else:
                nc.vector.tensor_copy(dst, src)
            ncopy[0] += 1

        # ---------------- GCN layers ----------------
        hv = hv1
        for l, (fi, fo) in enumerate(LAYERS):
            K = fi // 128
            nft = fo // 128
            wv = [wv1, wv2, wv3][l]
            if fo == 640:
                chunks = [(0, 0, 320), (320, 512, 320)]  # (m-col, psum-col, width)
            elif fo == 512:
                chunks = [(0, 0, 512)]
            else:
                chunks = [(0, 0, 256)]
            # lin: k-outer over nt-pairs so compute starts after first DMAs
            m16t = [big.tile([128, 640], DT16, tag=f"m16_{b}", name=f"m16_{l}_{b}") for b in range(NB)]
            for g0 in range(0, NB, 2):
                nts = [nt for nt in (g0, g0 + 1) if nt < NB]
                pss = {nt: psA.tile([128, 1536], DT32, tag="ps", name=f"lin{l}_{nt}", bufs=2) for nt in nts}
                for k in range(K):
                    ht, hb = hv[k]
                    wt, wb = wv[k]
                    for nt in nts:
                        for (mc, pc, w) in chunks:
                            nc.tensor.matmul(
                                pss[nt][:, pc : pc + w],
                                lhsT=ht[:, hb + nt * 128 : hb + (nt + 1) * 128],
                                rhs=wt[:, wb + mc : wb + mc + w],
                                start=(k == 0),
                                stop=(k == K - 1),
                            )
                for nt in nts:
                    for (mc, pc, w) in chunks:
                        ps_copy(m16t[nt][:, mc : mc + w], pss[nt][:, pc : pc + w])
            # scatter: s.T[f, dst] feature-major fp32 + stats
            sT = [big.tile([128, NT], DT32, tag=f"sT{ft}", name=f"sT{l}_{ft}") for ft in range(nft)]
            stats = big.tile([128, 2 * nft], DT32, tag=f"stats{l}", name=f"stats{l}")
            for ft in range(nft):
                pss = psA.tile([128, 1536], DT32, tag="ps", name=f"sc{l}_{ft}", bufs=2)
                for b in range(NB):
                    nc.tensor.matmul(
                        pss[:, b * 128 : (b + 1) * 128],
                        lhsT=m16t[b][:, ft * 128 : (ft + 1) * 128],
                        rhs=att[:, b * 128 : (b + 1) * 128],
                        start=(b % 4 == 0),
                        stop=(b in (3, 7, 8)),
                    )
                nc.scalar.activation(sT[ft][:, 0:PAD], pss[:, 0:PAD], AF.Copy)
                nc.scalar.activation(
                    sT[ft][:, PAD:NT], pss[:, PAD:NT], AF.Copy,
                    accum_out=stats[:, ft : ft + 1],
                )
                # sum of squares on vector (off the scalar engine)
                nc.vector.scalar_tensor_tensor(
                    sqs[:], sT[ft][:, PAD:NT], 1.0, sT[ft][:, PAD:NT],
                    ALU.mult, ALU.mult,
                    accum_out=stats[:, nft + ft : nft + ft + 1],
                )
            # allreduce stats; warm the Rsqrt table while it runs
            nc.sync.dma_start(out=cc_in[l][:], in_=stats[:])
            nc.scalar.activation(dumt[:], dumt[:], AF.Sqrt, bias=epst[:])
            nc.gpsimd.collective_compute(
                "AllReduce", ALU.add, replica_groups=rg,
                ins=[cc_in[l][:]], outs=[cc_out[l][:]],
            )
            statsg = big.tile([128, 2 * nft], DT32, tag=f"statsg{l}", name=f"statsg{l}")
            nc.sync.dma_start(out=statsg[:], in_=cc_out[l][:])
            # scale/bias
            mu = big.tile([128, nft], DT32, tag="mu", name=f"mu{l}")
            var = big.tile([128, nft], DT32, tag="var", name=f"var{l}")
            scl = big.tile([128, nft], DT32, tag="scl", name=f"scl{l}")
            bia = big.tile([128, nft], DT32, tag="bia", name=f"bia{l}")
            nc.vector.tensor_scalar_mul(mu[:], statsg[:, 0:nft], 1.0 / N_NODES)
            nc.vector.tensor_scalar_mul(var[:], statsg[:, nft : 2 * nft], 1.0 / N_NODES)
            nc.vector.tensor_mul(scl[:], mu[:], mu[:])
            nc.vector.tensor_sub(var[:], var[:], scl[:])
            nc.scalar.activation(var[:], var[:], AF.Sqrt, bias=epst[:])  # sqrt(var+eps)
            nc.scalar.activation(dumt[:], dumt[:], AF.Lrelu, alpha=0.01)  # warm Lrelu table
            nc.vector.reciprocal(var[:], var[:])
            nc.vector.tensor_mul(scl[:], misct[:, MC_G[l] : MC_G[l] + nft], var[:])
            nc.vector.tensor_mul(mu[:], mu[:], scl[:])
            nc.vector.tensor_sub(bia[:], misct[:, MC_BE[l] : MC_BE[l] + nft], mu[:])
            # apply + leaky -> next hT (fp16, feature-major)
            hTn = [big.tile([128, NT], DT16, tag=f"hT{l}_{ft}", name=f"hT{l}_{ft}") for ft in range(nft)]
            for ft in range(nft):
                nc.scalar.activation(
                    hTn[ft][:], sT[ft][:], AF.Lrelu,
                    bias=bia[:, ft : ft + 1], scale=scl[:, ft : ft + 1], alpha=0.01,
                )
            hv = [(hTn[ft], 0) for ft in range(nft)]

        # ---------------- pre-gates: PreT[m] = [gate, node] fp16 ----------------
        PreT = [big.tile([128, NT], DT16, tag=f"PreT{m}", name=f"PreT{m}") for m in range(8)]
        for m in range(8):
            psp = psA.tile([128, 1536], DT32, tag="ps", name=f"pre{m}", bufs=2)
            for k in range(2):
                for (n0, w) in [(0, 512), (512, 512), (1024, 128)]:
                    nc.tensor.matmul(
                        psp[:, n0 : n0 + w],
                        lhsT=wiht[:, (k * 8 + m) * 128 : (k * 8 + m + 1) * 128],
                        rhs=hv[k][0][:, n0 : n0 + w],
                        start=(k == 0),
                        stop=(k == 1),
                    )
            for (n0, w) in [(0, 512), (512, 512), (1024, 128)]:
                ps_copy(PreT[m][:, n0 : n0 + w], psp[:, n0 : n0 + w])
        # warm sigmoid/tanh table while PreO is built
        nc.scalar.activation(dumt[:], dumt[:], AF.Sigmoid)

        # ---------------- PreO: step-major pre-gates + bihh bias ------------
        # col = t*1024 + m*128 + c ; built from PreT with strided->contig copies
        PreO = big.tile([128, STEPS * 1024], DT16, tag="PreO", name="PreO")
        PreO3 = PreO[:].rearrange("p (tt x) -> p tt x", tt=STEPS)
        off0 = PAD - BURN
        neng = [0]

        def reorder_copy(dst, src, bias_col):
            if neng[0] % 2 == 0:
                nc.vector.tensor_scalar_add(dst, src, bias_col)
            else:
                nc.scalar.activation(dst, src, AF.Identity, bias=bias_col)
            neng[0] += 1

        for t0 in range(0, STEPS, 8):
            for m in range(8):
                src = PreT[m][:, off0 + t0 : off0 + t0 + 1024].rearrange(
                    "p (cc tt) -> p tt cc", tt=8)
                dst = PreO3[:, t0 : t0 + 8, m * 128 : (m + 1) * 128]
                reorder_copy(dst, src, misct[:, MC_BIHH + m : MC_BIHH + m + 1])

        psA_cm.__exit__(None, None, None)

        # ---------------- LSTM ----------------
        lsp = ctx.enter_context(tc.tile_pool(name="lsp", bufs=2))
        one = ctx.enter_context(tc.tile_pool(name="one", bufs=1))
        h_sb = one.tile([128, 2 * C], DT16, tag="h_sb", name="h_sb")
        c_sb = one.tile([128, 2 * C], DT32, tag="c_sb", name="c_sb")
        acc = one.tile([128, 2 * C], DT32, tag="acc", name="acc")
        nc.vector.memset(h_sb[:], 0.0)
        nc.vector.memset(c_sb[:], 0.0)
        nc.vector.memset(acc[:], 0.0)
        psB = ctx.enter_context(tc.tile_pool(name="psB", bufs=2, space="PSUM"))
        ident = wiht[:, 16 * 128 : 17 * 128]
        mask_at = {BURN - 1 - cc * LCH: cc for cc in range(4) if BURN - 1 - cc * LCH >= 0}
        for t in range(STEPS):
            gps = psB.tile([128, 1024], DT32, tag="gps", name="gps")
            nc.tensor.matmul(
                gps[:, 0:512], lhsT=ident,
                rhs=PreO[:, t * 1024 : t * 1024 + 512], start=True, stop=False)
            nc.tensor.matmul(
                gps[:, 512:1024], lhsT=ident,
                rhs=PreO[:, t * 1024 + 512 : (t + 1) * 1024], start=True, stop=False)
            sg = lsp.tile([128, 1024], DT32, tag="sg", name="sg")
            # m-blocks: 0,1=i  2,3=f  4,5=g  6,7=o ; g first (critical path)
            for m in (4, 5, 0, 1, 2, 3, 6, 7):
                for k in range(2):
                    nc.tensor.matmul(
                        gps[:, m * 128 : (m + 1) * 128],
                        lhsT=whht[:, (k * 8 + m) * 128 : (k * 8 + m + 1) * 128],
                        rhs=h_sb[:, k * C : (k + 1) * C],
                        start=False,
                        stop=(k == 1),
                    )
                if m == 5:
                    nc.scalar.activation(sg[:, 512:768], gps[:, 512:768], AF.Tanh)
                if m == 3:
                    nc.scalar.activation(sg[:, 0:512], gps[:, 0:512], AF.Sigmoid)
                if m == 7:
                    nc.scalar.activation(sg[:, 768:1024], gps[:, 768:1024], AF.Sigmoid)
            t1 = lsp.tile([128, 256], DT32, tag="t1", name="t1")
            pp = lsp.tile([128, 256], DT32, tag="pp", name="pp")
            th = lsp.tile([128, 256], DT32, tag="th", name="th")
            nc.vector.tensor_mul(t1[:], sg[:, 256:512], c_sb[:])       # f*c
            nc.vector.tensor_mul(pp[:], sg[:, 0:256], sg[:, 512:768])  # i*g
            nc.vector.tensor_add(c_sb[:], t1[:], pp[:])                # c
            nc.scalar.activation(th[:], c_sb[:], AF.Tanh)              # tanh(c)
            nc.vector.tensor_mul(h_sb[:], th[:], sg[:, 768:1024])      # h (fp16)
            if t >= BURN:
                nc.vector.tensor_add(acc[:], acc[:], h_sb[:])
            if t in mask_at:
                mi = mask_at[t]
                nc.vector.tensor_mul(h_sb[:], h_sb[:], msk16[:, mi * 2 * C : (mi + 1) * 2 * C])
                nc.vector.tensor_mul(c_sb[:], c_sb[:], misct[:, MC_MASK + mi * 2 * C : MC_MASK + (mi + 1) * 2 * C])

        # ---------------- pool + FC ----------------
        nc.scalar.activation(dumt[:], dumt[:], AF.Lrelu, alpha=0.01)  # warm Lrelu
        poolT = one.tile([128, 2, GPC], DT32, tag="poolT", name="poolT")
        accv = acc[:].rearrange("p (b g j) -> p b g j", b=2, g=GPC, j=LCH)
        nc.vector.tensor_reduce(poolT[:], accv, axis=mybir.AxisListType.X, op=ALU.add)
        fps = psB.tile([128, GPC], DT32, tag="fcps", name="fcps")
        for k in range(2):
            nc.tensor.matmul(fps[:], lhsT=fw1t[:, k * 128 : (k + 1) * 128], rhs=poolT[:, k, :], start=(k == 0), stop=(k == 1))
        fc1 = one.tile([128, GPC], DT32, tag="fc1", name="fc1")
        nc.scalar.activation(fc1[:], fps[:], AF.Lrelu, bias=misct[:, MC_FB1 : MC_FB1 + 1], alpha=0.01)
        fps2 = psB.tile([64, GPC], DT32, tag="fcps", name="fcps")
        nc.tensor.matmul(fps2[:], lhsT=fw2t[:], rhs=fc1[:], start=True, stop=True)
        fc2 = one.tile([64, GPC], DT32, tag="fc2", name="fc2")
        nc.scalar.activation(fc2[:], fps2[:], AF.Lrelu, bias=fb2t[:], alpha=0.01)
        fps3 = psB.tile([2, GPC], DT32, tag="fcps", name="fcps")
        nc.tensor.matmul(fps3[:], lhsT=fw3t[:], rhs=fc2[:], start=True, stop=True)
        fc3 = one.tile([2, GPC], DT32, tag="fc3", name="fc3")
        nc.scalar.activation(fc3[:], fps3[:], AF.Lrelu, bias=fb3t[:], alpha=0.01)
        nc.sync.dma_start(out=out_d[:], in_=fc3[:])

    return nc


def _prep_core(inputs, k, A):
    f16 = np.float16
    x = inputs["x"]
    lo, hi = k * NLOC - PAD, k * NLOC + NLOC
    xTk = np.zeros((1280, NT), f16)
    if k == 0:
        xTk[:, PAD : PAD + NLOC] = x[0:NLOC].T
    else:
        xTk[:, 0 : PAD + NLOC] = x[lo:hi].T
    ATk = np.zeros((NB, 128, 128), f16)
    glist = ([-1] if k == 0 else [k * GPC - 1]) + list(range(k * GPC, (k + 1) * GPC)) + [-1]
    for b in range(NB):
        ga, gb = glist[2 * b], glist[2 * b + 1]
        if ga >= 0:
            ATk[b, 0:64, 0:64] = A[ga].T
        if gb >= 0:
            ATk[b, 64:128, 64:128] = A[gb].T
    mk = np.ones((4, 2 * C), np.float32)
    if k == 0:
        for c in range(4):
            if BURN - 1 - c * LCH >= 0:
                mk[c, c] = 0.0
                mk[c, C + c] = 0.0

    def packk(w, kn, cols):  # [kn*128, cols] -> [128, kn*cols]
        out = np.zeros((128, kn * cols), w.dtype)
        for kk in range(kn):
            out[:, kk * cols : (kk + 1) * cols] = w[kk * 128 : (kk + 1) * 128, :]
        return out

    W1T = inputs["W1"].T.astype(f16)          # [1280, 640]
    W2T = inputs["W2"].T.astype(f16)          # [640, 512]
    W3T = inputs["W3"].T.astype(f16)          # [512, 256]
    WihT = inputs["Wih"].T.astype(f16)        # [256, 1024]
    WhhT = inputs["Whh"].T.astype(f16)        # [256, 1024]
    # wih/whh pack: [128, (k*8+m)*128 + c], plus identity appended to wih
    wih_p = np.zeros((128, 17 * 128), f16)
    whh_p = np.zeros((128, 16 * 128), f16)
    for kk in range(2):
        for m in range(8):
            wih_p[:, (kk * 8 + m) * 128 : (kk * 8 + m + 1) * 128] = WihT[kk * 128 : (kk + 1) * 128, m * 128 : (m + 1) * 128]
            whh_p[:, (kk * 8 + m) * 128 : (kk * 8 + m + 1) * 128] = WhhT[kk * 128 : (kk + 1) * 128, m * 128 : (m + 1) * 128]
    wih_p[:, 16 * 128 :] = np.eye(128, dtype=f16)

    misc = np.zeros((128, MISC_COLS), np.float32)
    for l, nft in enumerate((5, 4, 2)):
        misc[:, MC_G[l] : MC_G[l] + nft] = inputs[f"g{l+1}"].astype(np.float32).reshape(nft, 128).T
        misc[:, MC_BE[l] : MC_BE[l] + nft] = inputs[f"be{l+1}"].astype(np.float32).reshape(nft, 128).T
    misc[:, MC_BIHH : MC_BIHH + 8] = (inputs["bih"] + inputs["bhh"]).astype(np.float32).reshape(8, 128).T
    misc[:, MC_FB1] = inputs["fb1"].astype(np.float32)
    misc[:, MC_MASK : MC_MASK + 4 * 2 * C] = np.repeat(mk[None, :, :], 128, axis=0).reshape(128, -1)

    xp = packk(xTk, 10, NT)   # [128, 10*NT]
    im = {
        "w1a": packk(W1T, 10, 640)[:, 0:640].copy(),
        "w1b": packk(W1T, 10, 640)[:, 640:].copy(),
        "at": ATk.transpose(1, 0, 2).reshape(128, 9 * 128).copy(),
        "w2": packk(W2T, 5, 512),
        "w3": packk(W3T, 4, 256),
        "wih": wih_p,
        "whh": whh_p,
        "misc": misc,
        "fw1": packk(inputs["fW1"].T.astype(np.float32), 2, 128),
        "fw2": inputs["fW2"].T.astype(np.float32).copy(),
        "fw3": inputs["fW3"].T.astype(np.float32).copy(),
        "fb2": inputs["fb2"].astype(np.float32).reshape(64, 1),
        "fb3": inputs["fb3"].astype(np.float32).reshape(2, 1),
    }
    for i, (a, b) in enumerate(XGRP):
        im[f"xg{i}"] = xp[:, a * NT : b * NT].copy()
    return im


def kernel(**inputs):
    inputs = {k: np.asarray(v) for k, v in inputs.items()}
    src, dst = inputs["edge_index"][0], inputs["edge_index"][1]
    ew = inputs["edge_weight"].astype(np.float32)
    A = np.zeros((G, NPG, NPG), np.float32)
    np.add.at(A, (src // NPG, dst % NPG, src % NPG), ew)
    if "nc" not in _CACHE:
        _CACHE["nc"] = _build()
    nc = _CACHE["nc"]
    in_maps = [_prep_core(inputs, k, A) for k in range(NCORES)]
    res = run_bass_kernel_spmd(nc, in_maps, core_ids=list(range(NCORES)), **_CACHE.get("kw", {}))
    _CACHE["last"] = res
    out = np.zeros((G, 2), np.float32)
    for k in range(NCORES):
        out[k * GPC : (k + 1) * GPC, :] = res.results[k]["out"].T
    return out


# revision 11
# speedup vs baseline: 1.2410x; 1.0506x over previous
"""EEGGraphConvNetLSTM on 8 TRN2 NeuronCores (Bass/Tile).

Strategy: graph-level data parallel. Each core gets 16 graphs (1024 nodes)
plus a 64-node halo (previous core's last graph) used to burn in the LSTM
state. GCN message passing is done as dense block-diagonal [128x128]
adjacency matmuls (2 graphs per block). BatchNorm batch statistics are
all-reduced across cores. The 8192-step LSTM is run as 128 parallel chunks
of 8 steps per core, each chunk warmed up with B=16 burn-in steps.

v2: batched/ordered input DMAs, double-buffered scatter PSUM, sumsq on
gpsimd, Rsqrt/Lrelu activation-table prewarm during the all-reduce,
step-major pre-gate layout (PreO) so LSTM gathers are 2 fat contiguous
matmuls, tanh-based LSTM tail with gpsimd offload.
"""

import numpy as np
from contextlib import ExitStack

import concourse.bass as bass
import concourse.mybir as mybir
from concourse.tile import TileContext
from concourse.bass_utils import run_bass_kernel_spmd
from concourse.vector_clock import ScopedClock

# ---------------- walrus workaround: <=1 sync wait per instruction ----------
import concourse.tile as tile_mod


def _ap_dims_over2(ins):
    # >2-dim access patterns lower to S3D3 ISA structs that cannot carry
    # semaphore waits; their waits must be spilled to a preceding NOP.
    for a in list(getattr(ins, "ins", None) or []) + list(getattr(ins, "outs", None) or []):
        ap = getattr(a, "ap", None)
        if ap is not None and len(ap) > 2:
            return True
    return False


def _split_all_waits(nc):
    for _, b in list(nc.bb_map.items()):
        insts = b.bb.instructions
        out = []
        changed = False
        for ins in insts:
            si = getattr(ins, "sync_info", None)
            if si is not None and si.on_wait:
                spill_all = _ap_dims_over2(ins)
                if spill_all or len(si.on_wait) > 1:
                    waits = list(si.on_wait)
                    spill, keep = (waits, []) if spill_all else (waits[:-1], waits[-1:])
                    si.on_wait = keep
                    for w in spill:
                        nop = mybir.InstNoOp(
                            name=nc.get_next_instruction_name(), ins=[], outs=[]
                        )
                        nop.engine = ins.engine
                        nop.sync_info = mybir.SyncInfo(on_wait=[w], on_update=[])
                        nc.register_instruction(nop)
                        out.append(nop)
                    changed = True
            out.append(ins)
        if changed:
            b.bb.instructions[:] = out


def _patched_drain(self, tick_clock, wait_clock):
    nc = self.nc
    drain = nc.sync.drain()
    wait_clock.add_sem_waits(drain.ins, ScopedClock({None: tick_clock.global_clock}))
    nc.all_engine_barrier()
    assert self.sems is not None
    popped = nc._tile_sem_poison_stack.pop()
    assert popped is self._sem_poison
    nc.clear_and_free_semaphores(list(self.sems.allocated().values()))
    nc.all_engine_barrier()
    _split_all_waits(nc)


tile_mod.TileContext._drain_and_barrier = _patched_drain

# ---------------- constants ----------------
NCORES = 8
G, NPG = 128, 64          # graphs, nodes per graph
GPC = G // NCORES         # 16 graphs per core
NLOC = GPC * NPG          # 1024 own nodes
PAD = 64                  # halo (prev graph) + tail zero pad
NT = NLOC + 2 * PAD       # 1152 node columns per core
NB = NT // 128            # 9 two-graph blocks
LCH = 8                   # chunk length
C = 128                   # chunks per core
BURN = 16                 # LSTM burn-in steps
STEPS = BURN + LCH        # 24
H = 256
N_NODES = 8192

DT32 = mybir.dt.float32
DT16 = mybir.dt.float16
AF = mybir.ActivationFunctionType
ALU = mybir.AluOpType

LAYERS = [(1280, 640), (640, 512), (512, 256)]
# x tile k-groups per dram param: k0 | k1-2 | k3-5 | k6-9
XGRP = [(0, 1), (1, 3), (3, 6), (6, 10)]
# misc fp32 param column layout
MC_G = [0, 5, 9]          # g1,g2,g3
MC_BE = [11, 16, 20]      # be1,be2,be3
MC_BIHH = 22              # 8 cols
MC_FB1 = 30               # 1 col
MC_MASK = 32              # 4*256 cols
MISC_COLS = 32 + 4 * 2 * C

_CACHE = {}


def _build():
    nc = bass.Bass()
    # ---- dram params, packed to match SBUF tiles (few big DMAs)
    xg = [
        nc.declare_dram_parameter(f"xg{i}", [128, (b - a) * NT], DT16, isOutput=False)
        for i, (a, b) in enumerate(XGRP)
    ]
    w1a = nc.declare_dram_parameter("w1a", [128, 640], DT16, isOutput=False)
    w1b = nc.declare_dram_parameter("w1b", [128, 9 * 640], DT16, isOutput=False)
    at_d = nc.declare_dram_parameter("at", [128, 9 * 128], DT16, isOutput=False)
    w2_d = nc.declare_dram_parameter("w2", [128, 5 * 512], DT16, isOutput=False)
    w3_d = nc.declare_dram_parameter("w3", [128, 4 * 256], DT16, isOutput=False)
    wih_d = nc.declare_dram_parameter("wih", [128, 17 * 128], DT16, isOutput=False)
    whh_d = nc.declare_dram_parameter("whh", [128, 16 * 128], DT16, isOutput=False)
    misc_d = nc.declare_dram_parameter("misc", [128, MISC_COLS], DT32, isOutput=False)
    fw1_d = nc.declare_dram_parameter("fw1", [128, 256], DT32, isOutput=False)
    fw2_d = nc.declare_dram_parameter("fw2", [128, 64], DT32, isOutput=False)
    fw3_d = nc.declare_dram_parameter("fw3", [64, 2], DT32, isOutput=False)
    fb2_d = nc.declare_dram_parameter("fb2", [64, 1], DT32, isOutput=False)
    fb3_d = nc.declare_dram_parameter("fb3", [2, 1], DT32, isOutput=False)
    out_d = nc.declare_dram_parameter("out", [2, GPC], DT32, isOutput=True)

    cc_in = [nc.dram_tensor(f"cc_in{l}", [128, 2 * (LAYERS[l][1] // 128)], DT32) for l in range(3)]
    cc_out = [
        nc.dram_tensor(f"cc_out{l}", [128, 2 * (LAYERS[l][1] // 128)], DT32, addr_space="Shared")
        for l in range(3)
    ]
    rg = [list(range(NCORES))]
    cc_wi = nc.dram_tensor("cc_wi", [128, 1], DT32)
    cc_wo = nc.dram_tensor("cc_wo", [128, 1], DT32, addr_space="Shared")

    with TileContext(nc) as tc, ExitStack() as ctx:
        wp = ctx.enter_context(tc.tile_pool(name="wp", bufs=1))
        big = ctx.enter_context(tc.tile_pool(name="big", bufs=1))

        # ---- warmup collective (absorbs rendezvous) + scratch init
        warm = wp.tile([128, 1], DT32, tag="warm", name="warm")
        nc.vector.memset(warm[:], 0.0)
        nc.sync.dma_start(out=cc_wi[:], in_=warm[:])
        nc.gpsimd.collective_compute(
            "AllReduce", ALU.add, replica_groups=rg, ins=[cc_wi[:]], outs=[cc_wo[:]])
        dumt = wp.tile([128, 1], DT32, tag="dumt", name="dumt")
        nc.vector.memset(dumt[:], 1.0)
        epst = wp.tile([128, 1], DT32, tag="epst", name="epst")
        nc.vector.memset(epst[:], 1e-5)

        # ---- persistent weight/const tiles, ordered critical-first
        xt = []
        for i, (a, b) in enumerate(XGRP):
            t = wp.tile([128, (b - a) * NT], DT16, tag=f"xg{i}", name=f"xg{i}")
            xt.append(t)
        w1at = wp.tile([128, 640], DT16, tag="w1a", name="w1a")
        w1bt = wp.tile([128, 9 * 640], DT16, tag="w1b", name="w1b")
        att = wp.tile([128, 9 * 128], DT16, tag="at", name="at")
        w2t = wp.tile([128, 5 * 512], DT16, tag="w2", name="w2")
        w3t = wp.tile([128, 4 * 256], DT16, tag="w3", name="w3")
        wiht = wp.tile([128, 17 * 128], DT16, tag="wih", name="wih")
        whht = wp.tile([128, 16 * 128], DT16, tag="whh", name="whh")
        misct = wp.tile([128, MISC_COLS], DT32, tag="misc", name="misc")
        fw1t = wp.tile([128, 256], DT32, tag="fw1", name="fw1")
        fw2t = wp.tile([128, 64], DT32, tag="fw2", name="fw2")
        fw3t = wp.tile([64, 2], DT32, tag="fw3", name="fw3")
        fb2t = wp.tile([64, 1], DT32, tag="fb2", name="fb2")
        fb3t = wp.tile([2, 1], DT32, tag="fb3", name="fb3")

        nc.sync.dma_start(out=xt[0][:], in_=xg[0][:, :])
        nc.sync.dma_start(out=w1at[:], in_=w1a[:, :])
        nc.scalar.dma_start(out=xt[1][:], in_=xg[1][:, :])
        nc.scalar.dma_start(out=w1bt[:], in_=w1b[:, :])
        nc.gpsimd.dma_start(out=xt[2][:], in_=xg[2][:, :])
        nc.gpsimd.dma_start(out=xt[3][:], in_=xg[3][:, :])
        nc.sync.dma_start(out=att[:], in_=at_d[:, :])
        nc.sync.dma_start(out=w2t[:], in_=w2_d[:, :])
        nc.sync.dma_start(out=w3t[:], in_=w3_d[:, :])
        nc.sync.dma_start(out=wiht[:], in_=wih_d[:, :])
        nc.sync.dma_start(out=whht[:], in_=whh_d[:, :])
        nc.sync.dma_start(out=misct[:], in_=misc_d[:, :])
        nc.sync.dma_start(out=fw1t[:], in_=fw1_d[:, :])
        nc.sync.dma_start(out=fw2t[:], in_=fw2_d[:, :])
        nc.sync.dma_start(out=fw3t[:], in_=fw3_d[:, :])
        nc.sync.dma_start(out=fb2t[:], in_=fb2_d[:, :])
        nc.sync.dma_start(out=fb3t[:], in_=fb3_d[:, :])

        # fp16 masks derived on-chip
        msk16 = wp.tile([128, 4 * 2 * C], DT16, tag="msk16", name="msk16")
        nc.vector.tensor_copy(msk16[:], misct[:, MC_MASK : MC_MASK + 4 * 2 * C])

        # h-tile accessors: list of (tile, col_base) per k
        hv1 = []
        for i, (a, b) in enumerate(XGRP):
            for k in range(a, b):
                hv1.append((xt[i], (k - a) * NT))
        wv1 = [(w1at, 0)] + [(w1bt, (k - 1) * 640) for k in range(1, 10)]
        wv2 = [(w2t, k * 512) for k in range(5)]
        wv3 = [(w3t, k * 256) for k in range(4)]

        psA_cm = tc.tile_pool(name="psA", bufs=1, space="PSUM")
        psA = psA_cm.__enter__()

        sqs = big.tile([128, NT - PAD], DT32, tag="sqs", name="sqs")
        ncopy = [0]

        def ps_copy(dst, src):
            # rotate psum->sbuf copies between scalar and vector
            if ncopy[0] % 2 == 0:
                nc.scalar.activation(dst, src, AF.Copy)
            else:
                nc.vector.tensor_copy(dst, src)
            ncopy[0] += 1

        # ---------------- GCN layers ----------------
        hv = hv1
        for l, (fi, fo) in enumerate(LAYERS):
            K = fi // 128
            nft = fo // 128
            wv = [wv1, wv2, wv3][l]
            if fo == 640:
                chunks = [(0, 0, 320), (320, 512, 320)]  # (m-col, psum-col, width)
            elif fo == 512:
                chunks = [(0, 0, 512)]
            else:
                chunks = [(0, 0, 256)]
            # lin: k-outer over nt-pairs so compute starts after first DMAs
            m16t = [big.tile([128, 640], DT16, tag=f"m16_{b}", name=f"m16_{l}_{b}") for b in range(NB)]
            for g0 in range(0, NB, 2):
                nts = [nt for nt in (g0, g0 + 1) if nt < NB]
                pss = {nt: psA.tile([128, 1536], DT32, tag="ps", name=f"lin{l}_{nt}", bufs=2) for nt in nts}
                for k in range(K):
                    ht, hb = hv[k]
                    wt, wb = wv[k]
                    for nt in nts:
                        for (mc, pc, w) in chunks:
                            nc.tensor.matmul(
                                pss[nt][:, pc : pc + w],
                                lhsT=ht[:, hb + nt * 128 : hb + (nt + 1) * 128],
                                rhs=wt[:, wb + mc : wb + mc + w],
                                start=(k == 0),
                                stop=(k == K - 1),
                            )
                for nt in nts:
                    for (mc, pc, w) in chunks:
                        ps_copy(m16t[nt][:, mc : mc + w], pss[nt][:, pc : pc + w])
            # scatter: s.T[f, dst] feature-major fp32 + stats
            sT = [big.tile([128, NT], DT32, tag=f"sT{ft}", name=f"sT{l}_{ft}") for ft in range(nft)]
            stats = big.tile([128, 2 * nft], DT32, tag=f"stats{l}", name=f"stats{l}")
            for ft in range(nft):
                pss = psA.tile([128, 1536], DT32, tag="ps", name=f"sc{l}_{ft}", bufs=2)
                for b in range(NB):
                    nc.tensor.matmul(
                        pss[:, b * 128 : (b + 1) * 128],
                        lhsT=m16t[b][:, ft * 128 : (ft + 1) * 128],
                        rhs=att[:, b * 128 : (b + 1) * 128],
                        start=(b % 4 == 0),
                        stop=(b in (3, 7, 8)),
                    )
                nc.scalar.activation(sT[ft][:, 0:PAD], pss[:, 0:PAD], AF.Copy)
                nc.scalar.activation(
                    sT[ft][:, PAD:NT], pss[:, PAD:NT], AF.Copy,
                    accum_out=stats[:, ft : ft + 1],
                )
                # sum of squares on vector (off the scalar engine)
                nc.vector.scalar_tensor_tensor(
                    sqs[:], sT[ft][:, PAD:NT], 1.0, sT[ft][:, PAD:NT],
                    ALU.mult, ALU.mult,
                    accum_out=stats[:, nft + ft : nft + ft + 1],
                )
            # allreduce stats; warm the Rsqrt table while it runs
            nc.sync.dma_start(out=cc_in[l][:], in_=stats[:])
            nc.scalar.activation(dumt[:], dumt[:], AF.Sqrt, bias=epst[:])
            nc.gpsimd.collective_compute(
                "AllReduce", ALU.add, replica_groups=rg,
                ins=[cc_in[l][:]], outs=[cc_out[l][:]],
            )
            statsg = big.tile([128, 2 * nft], DT32, tag=f"statsg{l}", name=f"statsg{l}")
            nc.sync.dma_start(out=statsg[:], in_=cc_out[l][:])
            # scale/bias
            mu = big.tile([128, nft], DT32, tag="mu", name=f"mu{l}")
            var = big.tile([128, nft], DT32, tag="var", name=f"var{l}")
            scl = big.tile([128, nft], DT32, tag="scl", name=f"scl{l}")
            bia = big.tile([128, nft], DT32, tag="bia", name=f"bia{l}")
            nc.vector.tensor_scalar_mul(mu[:], statsg[:, 0:nft], 1.0 / N_NODES)
            nc.vector.tensor_scalar_mul(var[:], statsg[:, nft : 2 * nft], 1.0 / N_NODES)
            nc.vector.tensor_mul(scl[:], mu[:], mu[:])
            nc.vector.tensor_sub(var[:], var[:], scl[:])
            nc.scalar.activation(var[:], var[:], AF.Sqrt, bias=epst[:])  # sqrt(var+eps)
            nc.scalar.activation(dumt[:], dumt[:], AF.Lrelu, alpha=0.01)  # warm Lrelu table
            nc.vector.reciprocal(var[:], var[:])
            nc.vector.tensor_mul(scl[:], misct[:, MC_G[l] : MC_G[l] + nft], var[:])
            nc.vector.tensor_mul(mu[:], mu[:], scl[:])
            nc.vector.tensor_sub(bia[:], misct[:, MC_BE[l] : MC_BE[l] + nft], mu[:])
            # apply + leaky -> next hT (fp16, feature-major)
            hTn = [big.tile([128, NT], DT16, tag=f"hT{l}_{ft}", name=f"hT{l}_{ft}") for ft in range(nft)]
            for ft in range(nft):
                nc.scalar.activation(
                    hTn[ft][:], sT[ft][:], AF.Lrelu,
                    bias=bia[:, ft : ft + 1], scale=scl[:, ft : ft + 1], alpha=0.01,
                )
            hv = [(hTn[ft], 0) for ft in range(nft)]

        # ---------------- pre-gates: PreT[m] = [gate, node] fp16 ----------------
        PreT = [big.tile([128, NT], DT16, tag=f"PreT{m}", name=f"PreT{m}") for m in range(8)]
        for m in range(8):
            psp = psA.tile([128, 1536], DT32, tag="ps", name=f"pre{m}", bufs=2)
            for k in range(2):
                for (n0, w) in [(0, 512), (512, 512), (1024, 128)]:
                    nc.tensor.matmul(
                        psp[:, n0 : n0 + w],
                        lhsT=wiht[:, (k * 8 + m) * 128 : (k * 8 + m + 1) * 128],
                        rhs=hv[k][0][:, n0 : n0 + w],
                        start=(k == 0),
                        stop=(k == 1),
                    )
            for (n0, w) in [(0, 512), (512, 512), (1024, 128)]:
                ps_copy(PreT[m][:, n0 : n0 + w], psp[:, n0 : n0 + w])
        # warm sigmoid/tanh table while PreO is built
        nc.scalar.activation(dumt[:], dumt[:], AF.Sigmoid)

        # ---------------- PreO: step-major pre-gates + bihh bias ------------
        # col = t*1024 + m*128 + c ; built from PreT with strided->contig copies
        PreO = big.tile([128, STEPS * 1024], DT16, tag="PreO", name="PreO")
        PreO3 = PreO[:].rearrange("p (tt x) -> p tt x", tt=STEPS)
        off0 = PAD - BURN
        neng = [0]

        def reorder_copy(dst, src, bias_col):
            if neng[0] % 2 == 0:
                nc.vector.tensor_scalar_add(dst, src, bias_col)
            else:
                nc.scalar.activation(dst, src, AF.Identity, bias=bias_col)
            neng[0] += 1

        for t0 in range(0, STEPS, 8):
            for m in range(8):
                src = PreT[m][:, off0 + t0 : off0 + t0 + 1024].rearrange(
                    "p (cc tt) -> p tt cc", tt=8)
                dst = PreO3[:, t0 : t0 + 8, m * 128 : (m + 1) * 128]
                reorder_copy(dst, src, misct[:, MC_BIHH + m : MC_BIHH + m + 1])

        psA_cm.__exit__(None, None, None)

        # ---------------- LSTM ----------------
        lsp = ctx.enter_context(tc.tile_pool(name="lsp", bufs=2))
        one = ctx.enter_context(tc.tile_pool(name="one", bufs=1))
        h_sb = one.tile([128, 2 * C], DT16, tag="h_sb", name="h_sb")
        c_sb = one.tile([128, 2 * C], DT32, tag="c_sb", name="c_sb")
        acc = one.tile([128, 2 * C], DT32, tag="acc", name="acc")
        nc.vector.memset(h_sb[:], 0.0)
        nc.vector.memset(c_sb[:], 0.0)
        nc.vector.memset(acc[:], 0.0)
        psB = ctx.enter_context(tc.tile_pool(name="psB", bufs=2, space="PSUM"))
        ident = wiht[:, 16 * 128 : 17 * 128]
        mask_at = {BURN - 1 - cc * LCH: cc for cc in range(4) if BURN - 1 - cc * LCH >= 0}
        for t in range(STEPS):
            gA = psB.tile([128, 512], DT32, tag="gA", name="gA")  # i,f (m0-3)
            gB = psB.tile([128, 512], DT32, tag="gB", name="gB")  # g,o (m4-7)
            nc.tensor.matmul(
                gA[:], lhsT=ident,
                rhs=PreO[:, t * 1024 : t * 1024 + 512], start=True, stop=False)
            nc.tensor.matmul(
                gB[:], lhsT=ident,
                rhs=PreO[:, t * 1024 + 512 : (t + 1) * 1024], start=True, stop=False)
            sg = lsp.tile([128, 1024], DT32, tag="sg", name="sg")

            def whh_mm(m, k, tile, col):
                nc.tensor.matmul(
                    tile[:, col * 128 : (col + 1) * 128],
                    lhsT=whht[:, (k * 8 + m) * 128 : (k * 8 + m + 1) * 128],
                    rhs=h_sb[:, k * C : (k + 1) * C],
                    start=False, stop=(k == 1),
                )

            # g gates first (critical path), then i/f, then o
            for m in (4, 5):
                whh_mm(m, 0, gB, m - 4); whh_mm(m, 1, gB, m - 4)
            nc.scalar.activation(sg[:, 512:768], gB[:, 0:256], AF.Tanh)        # g
            for m in (0, 1, 2, 3):
                whh_mm(m, 0, gA, m); whh_mm(m, 1, gA, m)
            nc.scalar.activation(sg[:, 0:512], gA[:], AF.Sigmoid)              # i,f
            for m in (6, 7):
                whh_mm(m, 0, gB, m - 4); whh_mm(m, 1, gB, m - 4)
            nc.scalar.activation(sg[:, 768:1024], gB[:, 256:512], AF.Sigmoid)  # o
            t1 = lsp.tile([128, 256], DT32, tag="t1", name="t1")
            pp = lsp.tile([128, 256], DT32, tag="pp", name="pp")
            th = lsp.tile([128, 256], DT32, tag="th", name="th")
            for hh in (0, 1):
                hs = slice(hh * 128, (hh + 1) * 128)
                nc.vector.tensor_mul(pp[:, hs], sg[:, hh * 128 : (hh + 1) * 128], sg[:, 512 + hh * 128 : 512 + (hh + 1) * 128])
                nc.vector.tensor_mul(t1[:, hs], sg[:, 256 + hh * 128 : 256 + (hh + 1) * 128], c_sb[:, hs])
                nc.vector.tensor_add(c_sb[:, hs], t1[:, hs], pp[:, hs])
                nc.scalar.activation(th[:, hs], c_sb[:, hs], AF.Tanh)
            for hh in (0, 1):
                hs = slice(hh * 128, (hh + 1) * 128)
                nc.vector.tensor_mul(h_sb[:, hs], th[:, hs], sg[:, 768 + hh * 128 : 768 + (hh + 1) * 128])
            if t >= BURN:
                nc.vector.tensor_add(acc[:], acc[:], h_sb[:])
            if t in mask_at:
                mi = mask_at[t]
                nc.vector.tensor_mul(h_sb[:], h_sb[:], msk16[:, mi * 2 * C : (mi + 1) * 2 * C])
                nc.vector.tensor_mul(c_sb[:], c_sb[:], misct[:, MC_MASK + mi * 2 * C : MC_MASK + (mi + 1) * 2 * C])

        # ---------------- pool + FC ----------------
        nc.scalar.activation(dumt[:], dumt[:], AF.Lrelu, alpha=0.01)  # warm Lrelu
        poolT = one.tile([128, 2, GPC], DT32, tag="poolT", name="poolT")
        accv = acc[:].rearrange("p (b g j) -> p b g j", b=2, g=GPC, j=LCH)
        nc.vector.tensor_reduce(poolT[:], accv, axis=mybir.AxisListType.X, op=ALU.add)
        fps = psB.tile([128, GPC], DT32, tag="fcps", name="fcps")
        for k in range(2):
            nc.tensor.matmul(fps[:], lhsT=fw1t[:, k * 128 : (k + 1) * 128], rhs=poolT[:, k, :], start=(k == 0), stop=(k == 1))
        fc1 = one.tile([128, GPC], DT32, tag="fc1", name="fc1")
        nc.scalar.activation(fc1[:], fps[:], AF.Lrelu, bias=misct[:, MC_FB1 : MC_FB1 + 1], alpha=0.01)
        fps2 = psB.tile([64, GPC], DT32, tag="fcps", name="fcps")
        nc.tensor.matmul(fps2[:], lhsT=fw2t[:], rhs=fc1[:], start=True, stop=True)
        fc2 = one.tile([64, GPC], DT32, tag="fc2", name="fc2")
        nc.scalar.activation(fc2[:], fps2[:], AF.Lrelu, bias=fb2t[:], alpha=0.01)
        fps3 = psB.tile([2, GPC], DT32, tag="fcps", name="fcps")
        nc.tensor.matmul(fps3[:], lhsT=fw3t[:], rhs=fc2[:], start=True, stop=True)
        fc3 = one.tile([2, GPC], DT32, tag="fc3", name="fc3")
        nc.scalar.activation(fc3[:], fps3[:], AF.Lrelu, bias=fb3t[:], alpha=0.01)
        nc.sync.dma_start(out=out_d[:], in_=fc3[:])

    return nc


def _prep_core(inputs, k, A):
    f16 = np.float16
    x = inputs["x"]
    lo, hi = k * NLOC - PAD, k * NLOC + NLOC
    xTk = np.zeros((1280, NT), f16)
    if k == 0:
        xTk[:, PAD : PAD + NLOC] = x[0:NLOC].T
    else:
        xTk[:, 0 : PAD + NLOC] = x[lo:hi].T
    ATk = np.zeros((NB, 128, 128), f16)
    glist = ([-1] if k == 0 else [k * GPC - 1]) + list(range(k * GPC, (k + 1) * GPC)) + [-1]
    for b in range(NB):
        ga, gb = glist[2 * b], glist[2 * b + 1]
        if ga >= 0:
            ATk[b, 0:64, 0:64] = A[ga].T
        if gb >= 0:
            ATk[b, 64:128, 64:128] = A[gb].T
    mk = np.ones((4, 2 * C), np.float32)
    if k == 0:
        for c in range(4):
            if BURN - 1 - c * LCH >= 0:
                mk[c, c] = 0.0
                mk[c, C + c] = 0.0

    def packk(w, kn, cols):  # [kn*128, cols] -> [128, kn*cols]
        out = np.zeros((128, kn * cols), w.dtype)
        for kk in range(kn):
            out[:, kk * cols : (kk + 1) * cols] = w[kk * 128 : (kk + 1) * 128, :]
        return out

    W1T = inputs["W1"].T.astype(f16)          # [1280, 640]
    W2T = inputs["W2"].T.astype(f16)          # [640, 512]
    W3T = inputs["W3"].T.astype(f16)          # [512, 256]
    WihT = inputs["Wih"].T.astype(f16)        # [256, 1024]
    WhhT = inputs["Whh"].T.astype(f16)        # [256, 1024]
    # wih/whh pack: [128, (k*8+m)*128 + c], plus identity appended to wih
    wih_p = np.zeros((128, 17 * 128), f16)
    whh_p = np.zeros((128, 16 * 128), f16)
    for kk in range(2):
        for m in range(8):
            wih_p[:, (kk * 8 + m) * 128 : (kk * 8 + m + 1) * 128] = WihT[kk * 128 : (kk + 1) * 128, m * 128 : (m + 1) * 128]
            whh_p[:, (kk * 8 + m) * 128 : (kk * 8 + m + 1) * 128] = WhhT[kk * 128 : (kk + 1) * 128, m * 128 : (m + 1) * 128]
    wih_p[:, 16 * 128 :] = np.eye(128, dtype=f16)

    misc = np.zeros((128, MISC_COLS), np.float32)
    for l, nft in enumerate((5, 4, 2)):
        misc[:, MC_G[l] : MC_G[l] + nft] = inputs[f"g{l+1}"].astype(np.float32).reshape(nft, 128).T
        misc[:, MC_BE[l] : MC_BE[l] + nft] = inputs[f"be{l+1}"].astype(np.float32).reshape(nft, 128).T
    misc[:, MC_BIHH : MC_BIHH + 8] = (inputs["bih"] + inputs["bhh"]).astype(np.float32).reshape(8, 128).T
    misc[:, MC_FB1] = inputs["fb1"].astype(np.float32)
    misc[:, MC_MASK : MC_MASK + 4 * 2 * C] = np.repeat(mk[None, :, :], 128, axis=0).reshape(128, -1)

    xp = packk(xTk, 10, NT)   # [128, 10*NT]
    im = {
        "w1a": packk(W1T, 10, 640)[:, 0:640].copy(),
        "w1b": packk(W1T, 10, 640)[:, 640:].copy(),
        "at": ATk.transpose(1, 0, 2).reshape(128, 9 * 128).copy(),
        "w2": packk(W2T, 5, 512),
        "w3": packk(W3T, 4, 256),
        "wih": wih_p,
        "whh": whh_p,
        "misc": misc,
        "fw1": packk(inputs["fW1"].T.astype(np.float32), 2, 128),
        "fw2": inputs["fW2"].T.astype(np.float32).copy(),
        "fw3": inputs["fW3"].T.astype(np.float32).copy(),
        "fb2": inputs["fb2"].astype(np.float32).reshape(64, 1),
        "fb3": inputs["fb3"].astype(np.float32).reshape(2, 1),
    }
    for i, (a, b) in enumerate(XGRP):
        im[f"xg{i}"] = xp[:, a * NT : b * NT].copy()
    return im


def kernel(**inputs):
    inputs = {k: np.asarray(v) for k, v in inputs.items()}
    src, dst = inputs["edge_index"][0], inputs["edge_index"][1]
    ew = inputs["edge_weight"].astype(np.float32)
    A = np.zeros((G, NPG, NPG), np.float32)
    np.add.at(A, (src // NPG, dst % NPG, src % NPG), ew)
    if "nc" not in _CACHE:
        _CACHE["nc"] = _build()
    nc = _CACHE["nc"]
    in_maps = [_prep_core(inputs, k, A) for k in range(NCORES)]
    res = run_bass_kernel_spmd(nc, in_maps, core_ids=list(range(NCORES)), **_CACHE.get("kw", {}))
    _CACHE["last"] = res
    out = np.zeros((G, 2), np.float32)
    for k in range(NCORES):
        out[k * GPC : (k + 1) * GPC, :] = res.results[k]["out"].T
    return out


# revision 12
# speedup vs baseline: 1.2781x; 1.0299x over previous
"""EEGGraphConvNetLSTM on 8 TRN2 NeuronCores (Bass/Tile).

Strategy: graph-level data parallel. Each core gets 16 graphs (1024 nodes)
plus a 64-node halo (previous core's last graph) used to burn in the LSTM
state. GCN message passing is done as dense block-diagonal [128x128]
adjacency matmuls (2 graphs per block). BatchNorm batch statistics are
all-reduced across cores. The 8192-step LSTM is run as 128 parallel chunks
of 8 steps per core, each chunk warmed up with B=16 burn-in steps.

v2: batched/ordered input DMAs, double-buffered scatter PSUM, sumsq on
gpsimd, Rsqrt/Lrelu activation-table prewarm during the all-reduce,
step-major pre-gate layout (PreO) so LSTM gathers are 2 fat contiguous
matmuls, tanh-based LSTM tail with gpsimd offload.
"""

import numpy as np
from contextlib import ExitStack

import concourse.bass as bass
import concourse.mybir as mybir
from concourse.tile import TileContext
from concourse.bass_utils import run_bass_kernel_spmd
from concourse.vector_clock import ScopedClock

# ---------------- walrus workaround: <=1 sync wait per instruction ----------
import concourse.tile as tile_mod


def _ap_dims_over2(ins):
    # >2-dim access patterns lower to S3D3 ISA structs that cannot carry
    # semaphore waits; their waits must be spilled to a preceding NOP.
    for a in list(getattr(ins, "ins", None) or []) + list(getattr(ins, "outs", None) or []):
        ap = getattr(a, "ap", None)
        if ap is not None and len(ap) > 2:
            return True
    return False


def _split_all_waits(nc):
    for _, b in list(nc.bb_map.items()):
        insts = b.bb.instructions
        out = []
        changed = False
        for ins in insts:
            si = getattr(ins, "sync_info", None)
            if si is not None and si.on_wait:
                spill_all = _ap_dims_over2(ins)
                if spill_all or len(si.on_wait) > 1:
                    waits = list(si.on_wait)
                    spill, keep = (waits, []) if spill_all else (waits[:-1], waits[-1:])
                    si.on_wait = keep
                    for w in spill:
                        nop = mybir.InstNoOp(
                            name=nc.get_next_instruction_name(), ins=[], outs=[]
                        )
                        nop.engine = ins.engine
                        nop.sync_info = mybir.SyncInfo(on_wait=[w], on_update=[])
                        nc.register_instruction(nop)
                        out.append(nop)
                    changed = True
            out.append(ins)
        if changed:
            b.bb.instructions[:] = out


def _patched_drain(self, tick_clock, wait_clock):
    nc = self.nc
    drain = nc.sync.drain()
    wait_clock.add_sem_waits(drain.ins, ScopedClock({None: tick_clock.global_clock}))
    nc.all_engine_barrier()
    assert self.sems is not None
    popped = nc._tile_sem_poison_stack.pop()
    assert popped is self._sem_poison
    nc.clear_and_free_semaphores(list(self.sems.allocated().values()))
    nc.all_engine_barrier()
    _split_all_waits(nc)


tile_mod.TileContext._drain_and_barrier = _patched_drain

# ---------------- constants ----------------
NCORES = 8
G, NPG = 128, 64          # graphs, nodes per graph
GPC = G // NCORES         # 16 graphs per core
NLOC = GPC * NPG          # 1024 own nodes
PAD = 64                  # halo (prev graph) + tail zero pad
NT = NLOC + 2 * PAD       # 1152 node columns per core
NB = NT // 128            # 9 two-graph blocks
LCH = 8                   # chunk length
C = 128                   # chunks per core
BURN = 16                 # LSTM burn-in steps
STEPS = BURN + LCH        # 24
H = 256
N_NODES = 8192

DT32 = mybir.dt.float32
DT16 = mybir.dt.float16
AF = mybir.ActivationFunctionType
ALU = mybir.AluOpType

LAYERS = [(1280, 640), (640, 512), (512, 256)]
# x tile k-groups per dram param: k0 | k1-2 | k3-5 | k6-9
XGRP = [(0, 1), (1, 3), (3, 6), (6, 10)]
# misc fp32 param column layout
MC_G = [0, 5, 9]          # g1,g2,g3
MC_BE = [11, 16, 20]      # be1,be2,be3
MC_BIHH = 22              # 8 cols
MC_FB1 = 30               # 1 col
MC_MASK = 32              # 4*256 cols
MISC_COLS = 32 + 4 * 2 * C

_CACHE = {}


def _build():
    nc = bass.Bass()
    # ---- dram params, packed to match SBUF tiles (few big DMAs)
    xg = [
        nc.declare_dram_parameter(f"xg{i}", [128, (b - a) * NT], DT16, isOutput=False)
        for i, (a, b) in enumerate(XGRP)
    ]
    w1a = nc.declare_dram_parameter("w1a", [128, 640], DT16, isOutput=False)
    w1b = nc.declare_dram_parameter("w1b", [128, 9 * 640], DT16, isOutput=False)
    at_d = nc.declare_dram_parameter("at", [128, 9 * 128], DT16, isOutput=False)
    w2_d = nc.declare_dram_parameter("w2", [128, 5 * 512], DT16, isOutput=False)
    w3_d = nc.declare_dram_parameter("w3", [128, 4 * 256], DT16, isOutput=False)
    wih_d = nc.declare_dram_parameter("wih", [128, 17 * 128], DT16, isOutput=False)
    whh_d = nc.declare_dram_parameter("whh", [128, 16 * 128], DT16, isOutput=False)
    misc_d = nc.declare_dram_parameter("misc", [128, MISC_COLS], DT32, isOutput=False)
    fw1_d = nc.declare_dram_parameter("fw1", [128, 256], DT32, isOutput=False)
    fw2_d = nc.declare_dram_parameter("fw2", [128, 64], DT32, isOutput=False)
    fw3_d = nc.declare_dram_parameter("fw3", [64, 2], DT32, isOutput=False)
    fb2_d = nc.declare_dram_parameter("fb2", [64, 1], DT32, isOutput=False)
    fb3_d = nc.declare_dram_parameter("fb3", [2, 1], DT32, isOutput=False)
    out_d = nc.declare_dram_parameter("out", [2, GPC], DT32, isOutput=True)

    cc_in = [nc.dram_tensor(f"cc_in{l}", [128, 2 * (LAYERS[l][1] // 128)], DT32) for l in range(3)]
    cc_out = [
        nc.dram_tensor(f"cc_out{l}", [128, 2 * (LAYERS[l][1] // 128)], DT32, addr_space="Shared")
        for l in range(3)
    ]
    rg = [list(range(NCORES))]
    cc_wi = nc.dram_tensor("cc_wi", [128, 1], DT32)
    cc_wo = nc.dram_tensor("cc_wo", [128, 1], DT32, addr_space="Shared")

    with TileContext(nc) as tc, ExitStack() as ctx:
        wp = ctx.enter_context(tc.tile_pool(name="wp", bufs=1))
        big = ctx.enter_context(tc.tile_pool(name="big", bufs=1))

        # ---- warmup collective (absorbs rendezvous) + scratch init
        warm = wp.tile([128, 1], DT32, tag="warm", name="warm")
        nc.vector.memset(warm[:], 0.0)
        nc.sync.dma_start(out=cc_wi[:], in_=warm[:])
        nc.gpsimd.collective_compute(
            "AllReduce", ALU.add, replica_groups=rg, ins=[cc_wi[:]], outs=[cc_wo[:]])
        dumt = wp.tile([128, 1], DT32, tag="dumt", name="dumt")
        nc.vector.memset(dumt[:], 1.0)
        epst = wp.tile([128, 1], DT32, tag="epst", name="epst")
        nc.vector.memset(epst[:], 1e-5)

        # ---- persistent weight/const tiles, ordered critical-first
        xt = []
        for i, (a, b) in enumerate(XGRP):
            t = wp.tile([128, (b - a) * NT], DT16, tag=f"xg{i}", name=f"xg{i}")
            xt.append(t)
        w1at = wp.tile([128, 640], DT16, tag="w1a", name="w1a")
        w1bt = wp.tile([128, 9 * 640], DT16, tag="w1b", name="w1b")
        att = wp.tile([128, 9 * 128], DT16, tag="at", name="at")
        w2t = wp.tile([128, 5 * 512], DT16, tag="w2", name="w2")
        w3t = wp.tile([128, 4 * 256], DT16, tag="w3", name="w3")
        wiht = wp.tile([128, 17 * 128], DT16, tag="wih", name="wih")
        whht = wp.tile([128, 16 * 128], DT16, tag="whh", name="whh")
        misct = wp.tile([128, MISC_COLS], DT32, tag="misc", name="misc")
        fw1t = wp.tile([128, 256], DT32, tag="fw1", name="fw1")
        fw2t = wp.tile([128, 64], DT32, tag="fw2", name="fw2")
        fw3t = wp.tile([64, 2], DT32, tag="fw3", name="fw3")
        fb2t = wp.tile([64, 1], DT32, tag="fb2", name="fb2")
        fb3t = wp.tile([2, 1], DT32, tag="fb3", name="fb3")

        nc.sync.dma_start(out=xt[0][:], in_=xg[0][:, :])
        nc.sync.dma_start(out=w1at[:], in_=w1a[:, :])
        nc.sync.dma_start(out=xt[1][:], in_=xg[1][:, :])
        nc.sync.dma_start(out=w1bt[:], in_=w1b[:, :])
        nc.sync.dma_start(out=xt[2][:], in_=xg[2][:, :])
        nc.sync.dma_start(out=xt[3][:], in_=xg[3][:, :])
        nc.sync.dma_start(out=att[:], in_=at_d[:, :])
        nc.sync.dma_start(out=w2t[:], in_=w2_d[:, :])
        nc.sync.dma_start(out=w3t[:], in_=w3_d[:, :])
        nc.sync.dma_start(out=wiht[:], in_=wih_d[:, :])
        nc.sync.dma_start(out=whht[:], in_=whh_d[:, :])
        nc.sync.dma_start(out=misct[:], in_=misc_d[:, :])
        nc.sync.dma_start(out=fw1t[:], in_=fw1_d[:, :])
        nc.sync.dma_start(out=fw2t[:], in_=fw2_d[:, :])
        nc.sync.dma_start(out=fw3t[:], in_=fw3_d[:, :])
        nc.sync.dma_start(out=fb2t[:], in_=fb2_d[:, :])
        nc.sync.dma_start(out=fb3t[:], in_=fb3_d[:, :])

        # fp16 masks derived on-chip
        msk16 = wp.tile([128, 4 * 2 * C], DT16, tag="msk16", name="msk16")
        nc.vector.tensor_copy(msk16[:], misct[:, MC_MASK : MC_MASK + 4 * 2 * C])

        # h-tile accessors: list of (tile, col_base) per k
        hv1 = []
        for i, (a, b) in enumerate(XGRP):
            for k in range(a, b):
                hv1.append((xt[i], (k - a) * NT))
        wv1 = [(w1at, 0)] + [(w1bt, (k - 1) * 640) for k in range(1, 10)]
        wv2 = [(w2t, k * 512) for k in range(5)]
        wv3 = [(w3t, k * 256) for k in range(4)]

        psA_cm = tc.tile_pool(name="psA", bufs=1, space="PSUM")
        psA = psA_cm.__enter__()

        sqs = big.tile([128, NT - PAD], DT32, tag="sqs", name="sqs")
        ncopy = [0]

        def ps_copy(dst, src):
            # rotate psum->sbuf copies between scalar and vector
            if ncopy[0] % 2 == 0:
                nc.scalar.activation(dst, src, AF.Copy)
            else:
                nc.vector.tensor_copy(dst, src)
            ncopy[0] += 1

        # ---------------- GCN layers ----------------
        hv = hv1
        for l, (fi, fo) in enumerate(LAYERS):
            K = fi // 128
            nft = fo // 128
            wv = [wv1, wv2, wv3][l]
            if fo == 640:
                chunks = [(0, 0, 320), (320, 512, 320)]  # (m-col, psum-col, width)
            elif fo == 512:
                chunks = [(0, 0, 512)]
            else:
                chunks = [(0, 0, 256)]
            # lin: k-outer over nt-pairs so compute starts after first DMAs
            m16t = [big.tile([128, 640], DT16, tag=f"m16_{b}", name=f"m16_{l}_{b}") for b in range(NB)]
            for g0 in range(0, NB, 2):
                nts = [nt for nt in (g0, g0 + 1) if nt < NB]
                pss = {nt: psA.tile([128, 1536], DT32, tag="ps", name=f"lin{l}_{nt}", bufs=2) for nt in nts}
                for k in range(K):
                    ht, hb = hv[k]
                    wt, wb = wv[k]
                    for nt in nts:
                        for (mc, pc, w) in chunks:
                            nc.tensor.matmul(
                                pss[nt][:, pc : pc + w],
                                lhsT=ht[:, hb + nt * 128 : hb + (nt + 1) * 128],
                                rhs=wt[:, wb + mc : wb + mc + w],
                                start=(k == 0),
                                stop=(k == K - 1),
                            )
                for nt in nts:
                    for (mc, pc, w) in chunks:
                        ps_copy(m16t[nt][:, mc : mc + w], pss[nt][:, pc : pc + w])
            if l == 0:
                nc.sync.dma_start(out=cc_wi[:], in_=warm[:])
                nc.gpsimd.collective_compute(
                    "AllReduce", ALU.add, replica_groups=rg,
                    ins=[cc_wi[:]], outs=[cc_wo[:]])
            # scatter: s.T[f, dst] feature-major fp32 + stats
            sT = [big.tile([128, NT], DT32, tag=f"sT{ft}", name=f"sT{l}_{ft}") for ft in range(nft)]
            stats = big.tile([128, 2 * nft], DT32, tag=f"stats{l}", name=f"stats{l}")
            for ft in range(nft):
                pss = psA.tile([128, 1536], DT32, tag="ps", name=f"sc{l}_{ft}", bufs=2)
                for b in range(NB):
                    nc.tensor.matmul(
                        pss[:, b * 128 : (b + 1) * 128],
                        lhsT=m16t[b][:, ft * 128 : (ft + 1) * 128],
                        rhs=att[:, b * 128 : (b + 1) * 128],
                        start=(b % 4 == 0),
                        stop=(b in (3, 7, 8)),
                    )
                nc.scalar.activation(sT[ft][:, 0:PAD], pss[:, 0:PAD], AF.Copy)
                nc.scalar.activation(
                    sT[ft][:, PAD:NT], pss[:, PAD:NT], AF.Copy,
                    accum_out=stats[:, ft : ft + 1],
                )
                # sum of squares on vector (off the scalar engine)
                nc.vector.scalar_tensor_tensor(
                    sqs[:], sT[ft][:, PAD:NT], 1.0, sT[ft][:, PAD:NT],
                    ALU.mult, ALU.mult,
                    accum_out=stats[:, nft + ft : nft + ft + 1],
                )
            # allreduce stats; warm the Rsqrt table while it runs
            nc.sync.dma_start(out=cc_in[l][:], in_=stats[:])
            nc.scalar.activation(dumt[:], dumt[:], AF.Sqrt, bias=epst[:])
            nc.gpsimd.collective_compute(
                "AllReduce", ALU.add, replica_groups=rg,
                ins=[cc_in[l][:]], outs=[cc_out[l][:]],
            )
            statsg = big.tile([128, 2 * nft], DT32, tag=f"statsg{l}", name=f"statsg{l}")
            nc.sync.dma_start(out=statsg[:], in_=cc_out[l][:])
            # scale/bias
            mu = big.tile([128, nft], DT32, tag="mu", name=f"mu{l}")
            var = big.tile([128, nft], DT32, tag="var", name=f"var{l}")
            scl = big.tile([128, nft], DT32, tag="scl", name=f"scl{l}")
            bia = big.tile([128, nft], DT32, tag="bia", name=f"bia{l}")
            nc.vector.tensor_scalar_mul(mu[:], statsg[:, 0:nft], 1.0 / N_NODES)
            nc.vector.tensor_scalar_mul(var[:], statsg[:, nft : 2 * nft], 1.0 / N_NODES)
            nc.vector.tensor_mul(scl[:], mu[:], mu[:])
            nc.vector.tensor_sub(var[:], var[:], scl[:])
            nc.scalar.activation(var[:], var[:], AF.Sqrt, bias=epst[:])  # sqrt(var+eps)
            nc.scalar.activation(dumt[:], dumt[:], AF.Lrelu, alpha=0.01)  # warm Lrelu table
            nc.vector.reciprocal(var[:], var[:])
            nc.vector.tensor_mul(scl[:], misct[:, MC_G[l] : MC_G[l] + nft], var[:])
            nc.vector.tensor_mul(mu[:], mu[:], scl[:])
            nc.vector.tensor_sub(bia[:], misct[:, MC_BE[l] : MC_BE[l] + nft], mu[:])
            # apply + leaky -> next hT (fp16, feature-major)
            hTn = [big.tile([128, NT], DT16, tag=f"hT{l}_{ft}", name=f"hT{l}_{ft}") for ft in range(nft)]
            for ft in range(nft):
                nc.scalar.activation(
                    hTn[ft][:], sT[ft][:], AF.Lrelu,
                    bias=bia[:, ft : ft + 1], scale=scl[:, ft : ft + 1], alpha=0.01,
                )
            hv = [(hTn[ft], 0) for ft in range(nft)]

        # ---------------- pre-gates: PreT[m] = [gate, node] fp16 ----------------
        PreT = [big.tile([128, NT], DT16, tag=f"PreT{m}", name=f"PreT{m}") for m in range(8)]
        for m in range(8):
            psp = psA.tile([128, 1536], DT32, tag="ps", name=f"pre{m}", bufs=2)
            for k in range(2):
                for (n0, w) in [(0, 512), (512, 512), (1024, 128)]:
                    nc.tensor.matmul(
                        psp[:, n0 : n0 + w],
                        lhsT=wiht[:, (k * 8 + m) * 128 : (k * 8 + m + 1) * 128],
                        rhs=hv[k][0][:, n0 : n0 + w],
                        start=(k == 0),
                        stop=(k == 1),
                    )
            for (n0, w) in [(0, 512), (512, 512), (1024, 128)]:
                ps_copy(PreT[m][:, n0 : n0 + w], psp[:, n0 : n0 + w])
        # warm sigmoid/tanh table while PreO is built
        nc.scalar.activation(dumt[:], dumt[:], AF.Sigmoid)

        # ---------------- PreO: step-major pre-gates + bihh bias ------------
        # col = t*1024 + m*128 + c ; built from PreT with strided->contig copies
        PreO = big.tile([128, STEPS * 1024], DT16, tag="PreO", name="PreO")
        PreO3 = PreO[:].rearrange("p (tt x) -> p tt x", tt=STEPS)
        off0 = PAD - BURN
        neng = [0]

        def reorder_copy(dst, src, bias_col):
            if neng[0] % 2 == 0:
                nc.vector.tensor_scalar_add(dst, src, bias_col)
            else:
                nc.scalar.activation(dst, src, AF.Identity, bias=bias_col)
            neng[0] += 1

        for t0 in range(0, STEPS, 8):
            for m in range(8):
                src = PreT[m][:, off0 + t0 : off0 + t0 + 1024].rearrange(
                    "p (cc tt) -> p tt cc", tt=8)
                dst = PreO3[:, t0 : t0 + 8, m * 128 : (m + 1) * 128]
                reorder_copy(dst, src, misct[:, MC_BIHH + m : MC_BIHH + m + 1])

        psA_cm.__exit__(None, None, None)

        # ---------------- LSTM ----------------
        lsp = ctx.enter_context(tc.tile_pool(name="lsp", bufs=2))
        one = ctx.enter_context(tc.tile_pool(name="one", bufs=1))
        h_sb = one.tile([128, 2 * C], DT16, tag="h_sb", name="h_sb")
        c_sb = one.tile([128, 2 * C], DT32, tag="c_sb", name="c_sb")
        acc = one.tile([128, 2 * C], DT32, tag="acc", name="acc")
        nc.vector.memset(h_sb[:], 0.0)
        nc.vector.memset(c_sb[:], 0.0)
        nc.vector.memset(acc[:], 0.0)
        psB = ctx.enter_context(tc.tile_pool(name="psB", bufs=2, space="PSUM"))
        ident = wiht[:, 16 * 128 : 17 * 128]
        mask_at = {BURN - 1 - cc * LCH: cc for cc in range(4) if BURN - 1 - cc * LCH >= 0}
        for t in range(STEPS):
            gA = psB.tile([128, 512], DT32, tag="gA", name="gA")  # i,f (m0-3)
            gG = psB.tile([128, 256], DT32, tag="gG", name="gG")  # g (m4,5)
            gO = psB.tile([128, 256], DT32, tag="gO", name="gO")  # o (m6,7)
            nc.tensor.matmul(
                gG[:], lhsT=ident,
                rhs=PreO[:, t * 1024 + 512 : t * 1024 + 768], start=True, stop=False)
            nc.tensor.matmul(
                gO[:], lhsT=ident,
                rhs=PreO[:, t * 1024 + 768 : (t + 1) * 1024], start=True, stop=False)
            nc.tensor.matmul(
                gA[:], lhsT=ident,
                rhs=PreO[:, t * 1024 : t * 1024 + 512], start=True, stop=False)
            sg = lsp.tile([128, 1024], DT32, tag="sg", name="sg")

            def whh_mm(m, k, tile, col):
                nc.tensor.matmul(
                    tile[:, col * 128 : (col + 1) * 128],
                    lhsT=whht[:, (k * 8 + m) * 128 : (k * 8 + m + 1) * 128],
                    rhs=h_sb[:, k * C : (k + 1) * C],
                    start=False, stop=(k == 1),
                )

            # g gates first (critical path), then i/f, then o
            for m in (4, 5):
                whh_mm(m, 0, gG, m - 4); whh_mm(m, 1, gG, m - 4)
            nc.scalar.activation(sg[:, 512:768], gG[:], AF.Tanh)               # g
            for m in (0, 1, 2, 3):
                whh_mm(m, 0, gA, m); whh_mm(m, 1, gA, m)
            nc.scalar.activation(sg[:, 0:512], gA[:], AF.Sigmoid)              # i,f
            for m in (6, 7):
                whh_mm(m, 0, gO, m - 6); whh_mm(m, 1, gO, m - 6)
            nc.scalar.activation(sg[:, 768:1024], gO[:], AF.Sigmoid)           # o
            t1 = lsp.tile([128, 256], DT32, tag="t1", name="t1")
            pp = lsp.tile([128, 256], DT32, tag="pp", name="pp")
            th = lsp.tile([128, 256], DT32, tag="th", name="th")
            for hh in (0, 1):
                hs = slice(hh * 128, (hh + 1) * 128)
                nc.vector.tensor_mul(t1[:, hs], sg[:, 256 + hh * 128 : 256 + (hh + 1) * 128], c_sb[:, hs])
                nc.vector.tensor_mul(pp[:, hs], sg[:, hh * 128 : (hh + 1) * 128], sg[:, 512 + hh * 128 : 512 + (hh + 1) * 128])
                nc.vector.tensor_add(c_sb[:, hs], t1[:, hs], pp[:, hs])
                nc.scalar.activation(th[:, hs], c_sb[:, hs], AF.Tanh)
            for hh in (0, 1):
                hs = slice(hh * 128, (hh + 1) * 128)
                nc.vector.tensor_mul(h_sb[:, hs], th[:, hs], sg[:, 768 + hh * 128 : 768 + (hh + 1) * 128])
            if t >= BURN:
                nc.vector.tensor_add(acc[:], acc[:], h_sb[:])
            if t in mask_at:
                mi = mask_at[t]
                nc.vector.tensor_mul(h_sb[:], h_sb[:], msk16[:, mi * 2 * C : (mi + 1) * 2 * C])
                nc.vector.tensor_mul(c_sb[:], c_sb[:], misct[:, MC_MASK + mi * 2 * C : MC_MASK + (mi + 1) * 2 * C])

        # ---------------- pool + FC ----------------
        nc.scalar.activation(dumt[:], dumt[:], AF.Lrelu, alpha=0.01)  # warm Lrelu
        poolT = one.tile([128, 2, GPC], DT32, tag="poolT", name="poolT")
        accv = acc[:].rearrange("p (b g j) -> p b g j", b=2, g=GPC, j=LCH)
        nc.vector.tensor_reduce(poolT[:], accv, axis=mybir.AxisListType.X, op=ALU.add)
        fps = psB.tile([128, GPC], DT32, tag="fcps", name="fcps")
        for k in range(2):
            nc.tensor.matmul(fps[:], lhsT=fw1t[:, k * 128 : (k + 1) * 128], rhs=poolT[:, k, :], start=(k == 0), stop=(k == 1))
        fc1 = one.tile([128, GPC], DT32, tag="fc1", name="fc1")
        nc.scalar.activation(fc1[:], fps[:], AF.Lrelu, bias=misct[:, MC_FB1 : MC_FB1 + 1], alpha=0.01)
        fps2 = psB.tile([64, GPC], DT32, tag="fcps", name="fcps")
        nc.tensor.matmul(fps2[:], lhsT=fw2t[:], rhs=fc1[:], start=True, stop=True)
        fc2 = one.tile([64, GPC], DT32, tag="fc2", name="fc2")
        nc.scalar.activation(fc2[:], fps2[:], AF.Lrelu, bias=fb2t[:], alpha=0.01)
        fps3 = psB.tile([2, GPC], DT32, tag="fcps", name="fcps")
        nc.tensor.matmul(fps3[:], lhsT=fw3t[:], rhs=fc2[:], start=True, stop=True)
        fc3 = one.tile([2, GPC], DT32, tag="fc3", name="fc3")
        nc.scalar.activation(fc3[:], fps3[:], AF.Lrelu, bias=fb3t[:], alpha=0.01)
        nc.sync.dma_start(out=out_d[:], in_=fc3[:])

    return nc


def _prep_core(inputs, k, A):
    f16 = np.float16
    x = inputs["x"]
    lo, hi = k * NLOC - PAD, k * NLOC + NLOC
    xTk = np.zeros((1280, NT), f16)
    if k == 0:
        xTk[:, PAD : PAD + NLOC] = x[0:NLOC].T
    else:
        xTk[:, 0 : PAD + NLOC] = x[lo:hi].T
    ATk = np.zeros((NB, 128, 128), f16)
    glist = ([-1] if k == 0 else [k * GPC - 1]) + list(range(k * GPC, (k + 1) * GPC)) + [-1]
    for b in range(NB):
        ga, gb = glist[2 * b], glist[2 * b + 1]
        if ga >= 0:
            ATk[b, 0:64, 0:64] = A[ga].T
        if gb >= 0:
            ATk[b, 64:128, 64:128] = A[gb].T
    mk = np.ones((4, 2 * C), np.float32)
    if k == 0:
        for c in range(4):
            if BURN - 1 - c * LCH >= 0:
                mk[c, c] = 0.0
                mk[c, C + c] = 0.0

    def packk(w, kn, cols):  # [kn*128, cols] -> [128, kn*cols]
        out = np.zeros((128, kn * cols), w.dtype)
        for kk in range(kn):
            out[:, kk * cols : (kk + 1) * cols] = w[kk * 128 : (kk + 1) * 128, :]
        return out

    W1T = inputs["W1"].T.astype(f16)          # [1280, 640]
    W2T = inputs["W2"].T.astype(f16)          # [640, 512]
    W3T = inputs["W3"].T.astype(f16)          # [512, 256]
    WihT = inputs["Wih"].T.astype(f16)        # [256, 1024]
    WhhT = inputs["Whh"].T.astype(f16)        # [256, 1024]
    # wih/whh pack: [128, (k*8+m)*128 + c], plus identity appended to wih
    wih_p = np.zeros((128, 17 * 128), f16)
    whh_p = np.zeros((128, 16 * 128), f16)
    for kk in range(2):
        for m in range(8):
            wih_p[:, (kk * 8 + m) * 128 : (kk * 8 + m + 1) * 128] = WihT[kk * 128 : (kk + 1) * 128, m * 128 : (m + 1) * 128]
            whh_p[:, (kk * 8 + m) * 128 : (kk * 8 + m + 1) * 128] = WhhT[kk * 128 : (kk + 1) * 128, m * 128 : (m + 1) * 128]
    wih_p[:, 16 * 128 :] = np.eye(128, dtype=f16)

    misc = np.zeros((128, MISC_COLS), np.float32)
    for l, nft in enumerate((5, 4, 2)):
        misc[:, MC_G[l] : MC_G[l] + nft] = inputs[f"g{l+1}"].astype(np.float32).reshape(nft, 128).T
        misc[:, MC_BE[l] : MC_BE[l] + nft] = inputs[f"be{l+1}"].astype(np.float32).reshape(nft, 128).T
    misc[:, MC_BIHH : MC_BIHH + 8] = (inputs["bih"] + inputs["bhh"]).astype(np.float32).reshape(8, 128).T
    misc[:, MC_FB1] = inputs["fb1"].astype(np.float32)
    misc[:, MC_MASK : MC_MASK + 4 * 2 * C] = np.repeat(mk[None, :, :], 128, axis=0).reshape(128, -1)

    xp = packk(xTk, 10, NT)   # [128, 10*NT]
    im = {
        "w1a": packk(W1T, 10, 640)[:, 0:640].copy(),
        "w1b": packk(W1T, 10, 640)[:, 640:].copy(),
        "at": ATk.transpose(1, 0, 2).reshape(128, 9 * 128).copy(),
        "w2": packk(W2T, 5, 512),
        "w3": packk(W3T, 4, 256),
        "wih": wih_p,
        "whh": whh_p,
        "misc": misc,
        "fw1": packk(inputs["fW1"].T.astype(np.float32), 2, 128),
        "fw2": inputs["fW2"].T.astype(np.float32).copy(),
        "fw3": inputs["fW3"].T.astype(np.float32).copy(),
        "fb2": inputs["fb2"].astype(np.float32).reshape(64, 1),
        "fb3": inputs["fb3"].astype(np.float32).reshape(2, 1),
    }
    for i, (a, b) in enumerate(XGRP):
        im[f"xg{i}"] = xp[:, a * NT : b * NT].copy()
    return im


def kernel(**inputs):
    inputs = {k: np.asarray(v) for k, v in inputs.items()}
    src, dst = inputs["edge_index"][0], inputs["edge_index"][1]
    ew = inputs["edge_weight"].astype(np.float32)
    A = np.zeros((G, NPG, NPG), np.float32)
    np.add.at(A, (src // NPG, dst % NPG, src % NPG), ew)
    if "nc" not in _CACHE:
        _CACHE["nc"] = _build()
    nc = _CACHE["nc"]
    in_maps = [_prep_core(inputs, k, A) for k in range(NCORES)]
    res = run_bass_kernel_spmd(nc, in_maps, core_ids=list(range(NCORES)), **_CACHE.get("kw", {}))
    _CACHE["last"] = res
    out = np.zeros((G, 2), np.float32)
    for k in range(NCORES):
        out[k * GPC : (k + 1) * GPC, :] = res.results[k]["out"].T
    return out


# revision 13
# speedup vs baseline: 1.2839x; 1.0046x over previous
"""EEGGraphConvNetLSTM on 8 TRN2 NeuronCores (Bass/Tile).

Strategy: graph-level data parallel. Each core gets 16 graphs (1024 nodes)
plus a 64-node halo (previous core's last graph) used to burn in the LSTM
state. GCN message passing is done as dense block-diagonal [128x128]
adjacency matmuls (2 graphs per block). BatchNorm batch statistics are
all-reduced across cores. The 8192-step LSTM is run as 128 parallel chunks
of 8 steps per core, each chunk warmed up with B=16 burn-in steps.

v2: batched/ordered input DMAs, double-buffered scatter PSUM, sumsq on
gpsimd, Rsqrt/Lrelu activation-table prewarm during the all-reduce,
step-major pre-gate layout (PreO) so LSTM gathers are 2 fat contiguous
matmuls, tanh-based LSTM tail with gpsimd offload.
"""

import numpy as np
from contextlib import ExitStack

import concourse.bass as bass
import concourse.mybir as mybir
from concourse.tile import TileContext
from concourse.bass_utils import run_bass_kernel_spmd
from concourse.vector_clock import ScopedClock

# ---------------- walrus workaround: <=1 sync wait per instruction ----------
import concourse.tile as tile_mod


def _ap_dims_over2(ins):
    # >2-dim access patterns lower to S3D3 ISA structs that cannot carry
    # semaphore waits; their waits must be spilled to a preceding NOP.
    for a in list(getattr(ins, "ins", None) or []) + list(getattr(ins, "outs", None) or []):
        ap = getattr(a, "ap", None)
        if ap is not None and len(ap) > 2:
            return True
    return False


def _split_all_waits(nc):
    for _, b in list(nc.bb_map.items()):
        insts = b.bb.instructions
        out = []
        changed = False
        for ins in insts:
            si = getattr(ins, "sync_info", None)
            if si is not None and si.on_wait:
                spill_all = _ap_dims_over2(ins)
                if spill_all or len(si.on_wait) > 1:
                    waits = list(si.on_wait)
                    spill, keep = (waits, []) if spill_all else (waits[:-1], waits[-1:])
                    si.on_wait = keep
                    for w in spill:
                        nop = mybir.InstNoOp(
                            name=nc.get_next_instruction_name(), ins=[], outs=[]
                        )
                        nop.engine = ins.engine
                        nop.sync_info = mybir.SyncInfo(on_wait=[w], on_update=[])
                        nc.register_instruction(nop)
                        out.append(nop)
                    changed = True
            out.append(ins)
        if changed:
            b.bb.instructions[:] = out


def _patched_drain(self, tick_clock, wait_clock):
    nc = self.nc
    drain = nc.sync.drain()
    wait_clock.add_sem_waits(drain.ins, ScopedClock({None: tick_clock.global_clock}))
    nc.all_engine_barrier()
    assert self.sems is not None
    popped = nc._tile_sem_poison_stack.pop()
    assert popped is self._sem_poison
    nc.clear_and_free_semaphores(list(self.sems.allocated().values()))
    nc.all_engine_barrier()
    _split_all_waits(nc)


tile_mod.TileContext._drain_and_barrier = _patched_drain

# ---------------- constants ----------------
NCORES = 8
G, NPG = 128, 64          # graphs, nodes per graph
GPC = G // NCORES         # 16 graphs per core
NLOC = GPC * NPG          # 1024 own nodes
PAD = 64                  # halo (prev graph) + tail zero pad
NT = NLOC + 2 * PAD       # 1152 node columns per core
NB = NT // 128            # 9 two-graph blocks
LCH = 8                   # chunk length
C = 128                   # chunks per core
BURN = 16                 # LSTM burn-in steps
STEPS = BURN + LCH        # 24
H = 256
N_NODES = 8192

DT32 = mybir.dt.float32
DT16 = mybir.dt.float16
AF = mybir.ActivationFunctionType
ALU = mybir.AluOpType

LAYERS = [(1280, 640), (640, 512), (512, 256)]
# x tile k-groups per dram param: k0 | k1-2 | k3-5 | k6-9
XGRP = [(0, 1), (1, 3), (3, 6), (6, 10)]
# misc fp32 param column layout
MC_G = [0, 5, 9]          # g1,g2,g3
MC_BE = [11, 16, 20]      # be1,be2,be3
MC_BIHH = 22              # 8 cols
MC_FB1 = 30               # 1 col
MC_MASK = 32              # 4*256 cols
MISC_COLS = 32 + 4 * 2 * C

_CACHE = {}


def _build():
    nc = bass.Bass()
    # ---- dram params, packed to match SBUF tiles (few big DMAs)
    xg = [
        nc.declare_dram_parameter(f"xg{i}", [128, (b - a) * NT], DT16, isOutput=False)
        for i, (a, b) in enumerate(XGRP)
    ]
    w1a = nc.declare_dram_parameter("w1a", [128, 640], DT16, isOutput=False)
    w1b = nc.declare_dram_parameter("w1b", [128, 4 * 640], DT16, isOutput=False)
    w1c = nc.declare_dram_parameter("w1c", [128, 5 * 640], DT16, isOutput=False)
    at_d = nc.declare_dram_parameter("at", [128, 9 * 128], DT16, isOutput=False)
    w2_d = nc.declare_dram_parameter("w2", [128, 5 * 512], DT16, isOutput=False)
    w3_d = nc.declare_dram_parameter("w3", [128, 4 * 256], DT16, isOutput=False)
    wih_d = nc.declare_dram_parameter("wih", [128, 17 * 128], DT16, isOutput=False)
    whh_d = nc.declare_dram_parameter("whh", [128, 16 * 128], DT16, isOutput=False)
    misc_d = nc.declare_dram_parameter("misc", [128, MISC_COLS], DT32, isOutput=False)
    fw1_d = nc.declare_dram_parameter("fw1", [128, 256], DT32, isOutput=False)
    fw2_d = nc.declare_dram_parameter("fw2", [128, 64], DT32, isOutput=False)
    fw3_d = nc.declare_dram_parameter("fw3", [64, 2], DT32, isOutput=False)
    fb2_d = nc.declare_dram_parameter("fb2", [64, 1], DT32, isOutput=False)
    fb3_d = nc.declare_dram_parameter("fb3", [2, 1], DT32, isOutput=False)
    out_d = nc.declare_dram_parameter("out", [2, GPC], DT32, isOutput=True)

    cc_in = [nc.dram_tensor(f"cc_in{l}", [128, 2 * (LAYERS[l][1] // 128)], DT32) for l in range(3)]
    cc_out = [
        nc.dram_tensor(f"cc_out{l}", [128, 2 * (LAYERS[l][1] // 128)], DT32, addr_space="Shared")
        for l in range(3)
    ]
    rg = [list(range(NCORES))]
    cc_wi = nc.dram_tensor("cc_wi", [128, 1], DT32)
    cc_wo = nc.dram_tensor("cc_wo", [128, 1], DT32, addr_space="Shared")

    with TileContext(nc) as tc, ExitStack() as ctx:
        wp = ctx.enter_context(tc.tile_pool(name="wp", bufs=1))
        big = ctx.enter_context(tc.tile_pool(name="big", bufs=1))

        # ---- warmup collective (absorbs rendezvous) + scratch init
        warm = wp.tile([128, 1], DT32, tag="warm", name="warm")
        nc.vector.memset(warm[:], 0.0)
        nc.sync.dma_start(out=cc_wi[:], in_=warm[:])
        nc.gpsimd.collective_compute(
            "AllReduce", ALU.add, replica_groups=rg, ins=[cc_wi[:]], outs=[cc_wo[:]])
        dumt = wp.tile([128, 1], DT32, tag="dumt", name="dumt")
        nc.vector.memset(dumt[:], 1.0)
        epst = wp.tile([128, 1], DT32, tag="epst", name="epst")
        nc.vector.memset(epst[:], 1e-5)

        # ---- persistent weight/const tiles, ordered critical-first
        xt = []
        for i, (a, b) in enumerate(XGRP):
            t = wp.tile([128, (b - a) * NT], DT16, tag=f"xg{i}", name=f"xg{i}")
            xt.append(t)
        w1at = wp.tile([128, 640], DT16, tag="w1a", name="w1a")
        w1bt = wp.tile([128, 4 * 640], DT16, tag="w1b", name="w1b")
        w1ct = wp.tile([128, 5 * 640], DT16, tag="w1c", name="w1c")
        att = wp.tile([128, 9 * 128], DT16, tag="at", name="at")
        w2t = wp.tile([128, 5 * 512], DT16, tag="w2", name="w2")
        w3t = wp.tile([128, 4 * 256], DT16, tag="w3", name="w3")
        wiht = wp.tile([128, 17 * 128], DT16, tag="wih", name="wih")
        whht = wp.tile([128, 16 * 128], DT16, tag="whh", name="whh")
        misct = wp.tile([128, MISC_COLS], DT32, tag="misc", name="misc")
        fw1t = wp.tile([128, 256], DT32, tag="fw1", name="fw1")
        fw2t = wp.tile([128, 64], DT32, tag="fw2", name="fw2")
        fw3t = wp.tile([64, 2], DT32, tag="fw3", name="fw3")
        fb2t = wp.tile([64, 1], DT32, tag="fb2", name="fb2")
        fb3t = wp.tile([2, 1], DT32, tag="fb3", name="fb3")

        nc.sync.dma_start(out=xt[0][:], in_=xg[0][:, :])
        nc.sync.dma_start(out=w1at[:], in_=w1a[:, :])
        nc.sync.dma_start(out=xt[1][:], in_=xg[1][:, :])
        nc.sync.dma_start(out=w1bt[:], in_=w1b[:, :])
        nc.sync.dma_start(out=xt[2][:], in_=xg[2][:, :])
        nc.sync.dma_start(out=w1ct[:], in_=w1c[:, :])
        nc.sync.dma_start(out=xt[3][:], in_=xg[3][:, :])
        nc.sync.dma_start(out=att[:], in_=at_d[:, :])
        nc.sync.dma_start(out=w2t[:], in_=w2_d[:, :])
        nc.sync.dma_start(out=w3t[:], in_=w3_d[:, :])
        nc.sync.dma_start(out=wiht[:], in_=wih_d[:, :])
        nc.sync.dma_start(out=whht[:], in_=whh_d[:, :])
        nc.sync.dma_start(out=misct[:], in_=misc_d[:, :])
        nc.sync.dma_start(out=fw1t[:], in_=fw1_d[:, :])
        nc.sync.dma_start(out=fw2t[:], in_=fw2_d[:, :])
        nc.sync.dma_start(out=fw3t[:], in_=fw3_d[:, :])
        nc.sync.dma_start(out=fb2t[:], in_=fb2_d[:, :])
        nc.sync.dma_start(out=fb3t[:], in_=fb3_d[:, :])

        # fp16 masks derived on-chip
        msk16 = wp.tile([128, 4 * 2 * C], DT16, tag="msk16", name="msk16")
        nc.vector.tensor_copy(msk16[:], misct[:, MC_MASK : MC_MASK + 4 * 2 * C])

        # h-tile accessors: list of (tile, col_base) per k
        hv1 = []
        for i, (a, b) in enumerate(XGRP):
            for k in range(a, b):
                hv1.append((xt[i], (k - a) * NT))
        wv1 = ([(w1at, 0)] + [(w1bt, (k - 1) * 640) for k in range(1, 5)]
               + [(w1ct, (k - 5) * 640) for k in range(5, 10)])
        wv2 = [(w2t, k * 512) for k in range(5)]
        wv3 = [(w3t, k * 256) for k in range(4)]

        psA_cm = tc.tile_pool(name="psA", bufs=1, space="PSUM")
        psA = psA_cm.__enter__()

        sqs = big.tile([128, NT - PAD], DT32, tag="sqs", name="sqs")
        ncopy = [0]

        def ps_copy(dst, src):
            # rotate psum->sbuf copies between scalar and vector
            if ncopy[0] % 2 == 0:
                nc.scalar.activation(dst, src, AF.Copy)
            else:
                nc.vector.tensor_copy(dst, src)
            ncopy[0] += 1

        # ---------------- GCN layers ----------------
        hv = hv1
        for l, (fi, fo) in enumerate(LAYERS):
            K = fi // 128
            nft = fo // 128
            wv = [wv1, wv2, wv3][l]
            if fo == 640:
                chunks = [(0, 0, 320), (320, 512, 320)]  # (m-col, psum-col, width)
            elif fo == 512:
                chunks = [(0, 0, 512)]
            else:
                chunks = [(0, 0, 256)]
            # lin: k-outer over nt-pairs so compute starts after first DMAs
            m16t = [big.tile([128, 640], DT16, tag=f"m16_{b}", name=f"m16_{l}_{b}") for b in range(NB)]
            for g0 in range(0, NB, 2):
                nts = [nt for nt in (g0, g0 + 1) if nt < NB]
                pss = {nt: psA.tile([128, 1536], DT32, tag="ps", name=f"lin{l}_{nt}", bufs=2) for nt in nts}
                for k in range(K):
                    ht, hb = hv[k]
                    wt, wb = wv[k]
                    for nt in nts:
                        for (mc, pc, w) in chunks:
                            nc.tensor.matmul(
                                pss[nt][:, pc : pc + w],
                                lhsT=ht[:, hb + nt * 128 : hb + (nt + 1) * 128],
                                rhs=wt[:, wb + mc : wb + mc + w],
                                start=(k == 0),
                                stop=(k == K - 1),
                            )
                for nt in nts:
                    for (mc, pc, w) in chunks:
                        ps_copy(m16t[nt][:, mc : mc + w], pss[nt][:, pc : pc + w])
            # scatter: s.T[f, dst] feature-major fp32 + stats
            sT = [big.tile([128, NT], DT32, tag=f"sT{ft}", name=f"sT{l}_{ft}") for ft in range(nft)]
            stats = big.tile([128, 2 * nft], DT32, tag=f"stats{l}", name=f"stats{l}")
            for ft in range(nft):
                pss = psA.tile([128, 1536], DT32, tag="ps", name=f"sc{l}_{ft}", bufs=2)
                for b in range(NB):
                    nc.tensor.matmul(
                        pss[:, b * 128 : (b + 1) * 128],
                        lhsT=m16t[b][:, ft * 128 : (ft + 1) * 128],
                        rhs=att[:, b * 128 : (b + 1) * 128],
                        start=(b % 4 == 0),
                        stop=(b in (3, 7, 8)),
                    )
                nc.scalar.activation(sT[ft][:, 0:PAD], pss[:, 0:PAD], AF.Copy)
                nc.scalar.activation(
                    sT[ft][:, PAD:NT], pss[:, PAD:NT], AF.Copy,
                    accum_out=stats[:, ft : ft + 1],
                )
                # sum of squares on vector (off the scalar engine)
                nc.vector.scalar_tensor_tensor(
                    sqs[:], sT[ft][:, PAD:NT], 1.0, sT[ft][:, PAD:NT],
                    ALU.mult, ALU.mult,
                    accum_out=stats[:, nft + ft : nft + ft + 1],
                )
            # allreduce stats; warm the Rsqrt table while it runs
            nc.sync.dma_start(out=cc_in[l][:], in_=stats[:])
            nc.scalar.activation(dumt[:], dumt[:], AF.Sqrt, bias=epst[:])
            nc.gpsimd.collective_compute(
                "AllReduce", ALU.add, replica_groups=rg,
                ins=[cc_in[l][:]], outs=[cc_out[l][:]],
            )
            statsg = big.tile([128, 2 * nft], DT32, tag=f"statsg{l}", name=f"statsg{l}")
            nc.sync.dma_start(out=statsg[:], in_=cc_out[l][:])
            # scale/bias
            mu = big.tile([128, nft], DT32, tag="mu", name=f"mu{l}")
            var = big.tile([128, nft], DT32, tag="var", name=f"var{l}")
            scl = big.tile([128, nft], DT32, tag="scl", name=f"scl{l}")
            bia = big.tile([128, nft], DT32, tag="bia", name=f"bia{l}")
            nc.vector.tensor_scalar_mul(mu[:], statsg[:, 0:nft], 1.0 / N_NODES)
            nc.vector.tensor_scalar_mul(var[:], statsg[:, nft : 2 * nft], 1.0 / N_NODES)
            nc.vector.tensor_mul(scl[:], mu[:], mu[:])
            nc.vector.tensor_sub(var[:], var[:], scl[:])
            nc.scalar.activation(var[:], var[:], AF.Sqrt, bias=epst[:])  # sqrt(var+eps)
            nc.scalar.activation(dumt[:], dumt[:], AF.Lrelu, alpha=0.01)  # warm Lrelu table
            nc.vector.reciprocal(var[:], var[:])
            nc.vector.tensor_mul(scl[:], misct[:, MC_G[l] : MC_G[l] + nft], var[:])
            nc.vector.tensor_mul(mu[:], mu[:], scl[:])
            nc.vector.tensor_sub(bia[:], misct[:, MC_BE[l] : MC_BE[l] + nft], mu[:])
            # apply + leaky -> next hT (fp16, feature-major)
            hTn = [big.tile([128, NT], DT16, tag=f"hT{l}_{ft}", name=f"hT{l}_{ft}") for ft in range(nft)]
            for ft in range(nft):
                nc.scalar.activation(
                    hTn[ft][:], sT[ft][:], AF.Lrelu,
                    bias=bia[:, ft : ft + 1], scale=scl[:, ft : ft + 1], alpha=0.01,
                )
            hv = [(hTn[ft], 0) for ft in range(nft)]

        # ---------------- pre-gates: PreT[m] = [gate, node] fp16 ----------------
        PreT = [big.tile([128, NT], DT16, tag=f"PreT{m}", name=f"PreT{m}") for m in range(8)]
        for m in range(8):
            psp = psA.tile([128, 1536], DT32, tag="ps", name=f"pre{m}", bufs=2)
            for k in range(2):
                for (n0, w) in [(0, 512), (512, 512), (1024, 128)]:
                    nc.tensor.matmul(
                        psp[:, n0 : n0 + w],
                        lhsT=wiht[:, (k * 8 + m) * 128 : (k * 8 + m + 1) * 128],
                        rhs=hv[k][0][:, n0 : n0 + w],
                        start=(k == 0),
                        stop=(k == 1),
                    )
            for (n0, w) in [(0, 512), (512, 512), (1024, 128)]:
                ps_copy(PreT[m][:, n0 : n0 + w], psp[:, n0 : n0 + w])
        # warm sigmoid/tanh table while PreO is built
        nc.scalar.activation(dumt[:], dumt[:], AF.Sigmoid)

        # ---------------- PreO: step-major pre-gates + bihh bias ------------
        # col = t*1024 + m*128 + c ; built from PreT with strided->contig copies
        PreO = big.tile([128, STEPS * 1024], DT16, tag="PreO", name="PreO")
        PreO3 = PreO[:].rearrange("p (tt x) -> p tt x", tt=STEPS)
        off0 = PAD - BURN
        neng = [0]

        def reorder_copy(dst, src, bias_col):
            if neng[0] % 2 == 0:
                nc.vector.tensor_scalar_add(dst, src, bias_col)
            else:
                nc.scalar.activation(dst, src, AF.Identity, bias=bias_col)
            neng[0] += 1

        for t0 in range(0, STEPS, 8):
            for m in range(8):
                src = PreT[m][:, off0 + t0 : off0 + t0 + 1024].rearrange(
                    "p (cc tt) -> p tt cc", tt=8)
                dst = PreO3[:, t0 : t0 + 8, m * 128 : (m + 1) * 128]
                reorder_copy(dst, src, misct[:, MC_BIHH + m : MC_BIHH + m + 1])

        psA_cm.__exit__(None, None, None)

        # ---------------- LSTM ----------------
        lsp = ctx.enter_context(tc.tile_pool(name="lsp", bufs=2))
        one = ctx.enter_context(tc.tile_pool(name="one", bufs=1))
        h_sb = one.tile([128, 2 * C], DT16, tag="h_sb", name="h_sb")
        c_sb = one.tile([128, 2 * C], DT32, tag="c_sb", name="c_sb")
        acc = one.tile([128, 2 * C], DT32, tag="acc", name="acc")
        nc.vector.memset(h_sb[:], 0.0)
        nc.vector.memset(c_sb[:], 0.0)
        nc.vector.memset(acc[:], 0.0)
        psB = ctx.enter_context(tc.tile_pool(name="psB", bufs=2, space="PSUM"))
        ident = wiht[:, 16 * 128 : 17 * 128]
        mask_at = {BURN - 1 - cc * LCH: cc for cc in range(4) if BURN - 1 - cc * LCH >= 0}
        for t in range(STEPS):
            gA = psB.tile([128, 512], DT32, tag="gA", name="gA")  # i,f (m0-3)
            gG = psB.tile([128, 256], DT32, tag="gG", name="gG")  # g (m4,5)
            gO = psB.tile([128, 256], DT32, tag="gO", name="gO")  # o (m6,7)
            nc.tensor.matmul(
                gG[:], lhsT=ident,
                rhs=PreO[:, t * 1024 + 512 : t * 1024 + 768], start=True, stop=False)
            nc.tensor.matmul(
                gO[:], lhsT=ident,
                rhs=PreO[:, t * 1024 + 768 : (t + 1) * 1024], start=True, stop=False)
            nc.tensor.matmul(
                gA[:], lhsT=ident,
                rhs=PreO[:, t * 1024 : t * 1024 + 512], start=True, stop=False)
            sg = lsp.tile([128, 1024], DT16, tag="sg", name="sg")

            def whh_mm(m, k, tile, col):
                nc.tensor.matmul(
                    tile[:, col * 128 : (col + 1) * 128],
                    lhsT=whht[:, (k * 8 + m) * 128 : (k * 8 + m + 1) * 128],
                    rhs=h_sb[:, k * C : (k + 1) * C],
                    start=False, stop=(k == 1),
                )

            # g gates first (critical path), then i/f, then o
            for m in (4, 5):
                whh_mm(m, 0, gG, m - 4); whh_mm(m, 1, gG, m - 4)
            nc.scalar.activation(sg[:, 512:768], gG[:], AF.Tanh)               # g
            for m in (0, 1, 2, 3):
                whh_mm(m, 0, gA, m); whh_mm(m, 1, gA, m)
            nc.scalar.activation(sg[:, 0:512], gA[:], AF.Sigmoid)              # i,f
            for m in (6, 7):
                whh_mm(m, 0, gO, m - 6); whh_mm(m, 1, gO, m - 6)
            nc.scalar.activation(sg[:, 768:1024], gO[:], AF.Sigmoid)           # o
            t1 = lsp.tile([128, 256], DT32, tag="t1", name="t1")
            pp = lsp.tile([128, 256], DT32, tag="pp", name="pp")
            th = lsp.tile([128, 256], DT16, tag="th", name="th")
            for hh in (0, 1):
                hs = slice(hh * 128, (hh + 1) * 128)
                nc.vector.tensor_mul(t1[:, hs], sg[:, 256 + hh * 128 : 256 + (hh + 1) * 128], c_sb[:, hs])
                nc.vector.tensor_mul(pp[:, hs], sg[:, hh * 128 : (hh + 1) * 128], sg[:, 512 + hh * 128 : 512 + (hh + 1) * 128])
                nc.vector.tensor_add(c_sb[:, hs], t1[:, hs], pp[:, hs])
                nc.scalar.activation(th[:, hs], c_sb[:, hs], AF.Tanh)
            for hh in (0, 1):
                hs = slice(hh * 128, (hh + 1) * 128)
                nc.vector.tensor_mul(h_sb[:, hs], th[:, hs], sg[:, 768 + hh * 128 : 768 + (hh + 1) * 128])
            if t >= BURN:
                nc.vector.tensor_add(acc[:], acc[:], h_sb[:])
            if t in mask_at:
                mi = mask_at[t]
                nc.vector.tensor_mul(h_sb[:], h_sb[:], msk16[:, mi * 2 * C : (mi + 1) * 2 * C])
                nc.vector.tensor_mul(c_sb[:], c_sb[:], misct[:, MC_MASK + mi * 2 * C : MC_MASK + (mi + 1) * 2 * C])

        # ---------------- pool + FC ----------------
        nc.scalar.activation(dumt[:], dumt[:], AF.Lrelu, alpha=0.01)  # warm Lrelu
        poolT = one.tile([128, 2, GPC], DT32, tag="poolT", name="poolT")
        accv = acc[:].rearrange("p (b g j) -> p b g j", b=2, g=GPC, j=LCH)
        nc.vector.tensor_reduce(poolT[:], accv, axis=mybir.AxisListType.X, op=ALU.add)
        fps = psB.tile([128, GPC], DT32, tag="fcps", name="fcps")
        for k in range(2):
            nc.tensor.matmul(fps[:], lhsT=fw1t[:, k * 128 : (k + 1) * 128], rhs=poolT[:, k, :], start=(k == 0), stop=(k == 1))
        fc1 = one.tile([128, GPC], DT32, tag="fc1", name="fc1")
        nc.scalar.activation(fc1[:], fps[:], AF.Lrelu, bias=misct[:, MC_FB1 : MC_FB1 + 1], alpha=0.01)
        fps2 = psB.tile([64, GPC], DT32, tag="fcps", name="fcps")
        nc.tensor.matmul(fps2[:], lhsT=fw2t[:], rhs=fc1[:], start=True, stop=True)
        fc2 = one.tile([64, GPC], DT32, tag="fc2", name="fc2")
        nc.scalar.activation(fc2[:], fps2[:], AF.Lrelu, bias=fb2t[:], alpha=0.01)
        fps3 = psB.tile([2, GPC], DT32, tag="fcps", name="fcps")
        nc.tensor.matmul(fps3[:], lhsT=fw3t[:], rhs=fc2[:], start=True, stop=True)
        fc3 = one.tile([2, GPC], DT32, tag="fc3", name="fc3")
        nc.scalar.activation(fc3[:], fps3[:], AF.Lrelu, bias=fb3t[:], alpha=0.01)
        nc.sync.dma_start(out=out_d[:], in_=fc3[:])

    return nc


def _prep_core(inputs, k, A):
    f16 = np.float16
    x = inputs["x"]
    lo, hi = k * NLOC - PAD, k * NLOC + NLOC
    xTk = np.zeros((1280, NT), f16)
    if k == 0:
        xTk[:, PAD : PAD + NLOC] = x[0:NLOC].T
    else:
        xTk[:, 0 : PAD + NLOC] = x[lo:hi].T
    ATk = np.zeros((NB, 128, 128), f16)
    glist = ([-1] if k == 0 else [k * GPC - 1]) + list(range(k * GPC, (k + 1) * GPC)) + [-1]
    for b in range(NB):
        ga, gb = glist[2 * b], glist[2 * b + 1]
        if ga >= 0:
            ATk[b, 0:64, 0:64] = A[ga].T
        if gb >= 0:
            ATk[b, 64:128, 64:128] = A[gb].T
    mk = np.ones((4, 2 * C), np.float32)
    if k == 0:
        for c in range(4):
            if BURN - 1 - c * LCH >= 0:
                mk[c, c] = 0.0
                mk[c, C + c] = 0.0

    def packk(w, kn, cols):  # [kn*128, cols] -> [128, kn*cols]
        out = np.zeros((128, kn * cols), w.dtype)
        for kk in range(kn):
            out[:, kk * cols : (kk + 1) * cols] = w[kk * 128 : (kk + 1) * 128, :]
        return out

    W1T = inputs["W1"].T.astype(f16)          # [1280, 640]
    W2T = inputs["W2"].T.astype(f16)          # [640, 512]
    W3T = inputs["W3"].T.astype(f16)          # [512, 256]
    WihT = inputs["Wih"].T.astype(f16)        # [256, 1024]
    WhhT = inputs["Whh"].T.astype(f16)        # [256, 1024]
    # wih/whh pack: [128, (k*8+m)*128 + c], plus identity appended to wih
    wih_p = np.zeros((128, 17 * 128), f16)
    whh_p = np.zeros((128, 16 * 128), f16)
    for kk in range(2):
        for m in range(8):
            wih_p[:, (kk * 8 + m) * 128 : (kk * 8 + m + 1) * 128] = WihT[kk * 128 : (kk + 1) * 128, m * 128 : (m + 1) * 128]
            whh_p[:, (kk * 8 + m) * 128 : (kk * 8 + m + 1) * 128] = WhhT[kk * 128 : (kk + 1) * 128, m * 128 : (m + 1) * 128]
    wih_p[:, 16 * 128 :] = np.eye(128, dtype=f16)

    misc = np.zeros((128, MISC_COLS), np.float32)
    for l, nft in enumerate((5, 4, 2)):
        misc[:, MC_G[l] : MC_G[l] + nft] = inputs[f"g{l+1}"].astype(np.float32).reshape(nft, 128).T
        misc[:, MC_BE[l] : MC_BE[l] + nft] = inputs[f"be{l+1}"].astype(np.float32).reshape(nft, 128).T
    misc[:, MC_BIHH : MC_BIHH + 8] = (inputs["bih"] + inputs["bhh"]).astype(np.float32).reshape(8, 128).T
    misc[:, MC_FB1] = inputs["fb1"].astype(np.float32)
    misc[:, MC_MASK : MC_MASK + 4 * 2 * C] = np.repeat(mk[None, :, :], 128, axis=0).reshape(128, -1)

    xp = packk(xTk, 10, NT)   # [128, 10*NT]
    im = {
        "w1a": packk(W1T, 10, 640)[:, 0:640].copy(),
        "w1b": packk(W1T, 10, 640)[:, 640:3200].copy(),
        "w1c": packk(W1T, 10, 640)[:, 3200:].copy(),
        "at": ATk.transpose(1, 0, 2).reshape(128, 9 * 128).copy(),
        "w2": packk(W2T, 5, 512),
        "w3": packk(W3T, 4, 256),
        "wih": wih_p,
        "whh": whh_p,
        "misc": misc,
        "fw1": packk(inputs["fW1"].T.astype(np.float32), 2, 128),
        "fw2": inputs["fW2"].T.astype(np.float32).copy(),
        "fw3": inputs["fW3"].T.astype(np.float32).copy(),
        "fb2": inputs["fb2"].astype(np.float32).reshape(64, 1),
        "fb3": inputs["fb3"].astype(np.float32).reshape(2, 1),
    }
    for i, (a, b) in enumerate(XGRP):
        im[f"xg{i}"] = xp[:, a * NT : b * NT].copy()
    return im


def kernel(**inputs):
    inputs = {k: np.asarray(v) for k, v in inputs.items()}
    src, dst = inputs["edge_index"][0], inputs["edge_index"][1]
    ew = inputs["edge_weight"].astype(np.float32)
    A = np.zeros((G, NPG, NPG), np.float32)
    np.add.at(A, (src // NPG, dst % NPG, src % NPG), ew)
    if "nc" not in _CACHE:
        _CACHE["nc"] = _build()
    nc = _CACHE["nc"]
    in_maps = [_prep_core(inputs, k, A) for k in range(NCORES)]
    res = run_bass_kernel_spmd(nc, in_maps, core_ids=list(range(NCORES)), **_CACHE.get("kw", {}))
    _CACHE["last"] = res
    out = np.zeros((G, 2), np.float32)
    for k in range(NCORES):
        out[k * GPC : (k + 1) * GPC, :] = res.results[k]["out"].T
    return out
